# revision 19
# baseline (speedup 1.0000x reference)
"""Trainium2 Bass kernel for a 6-layer transformer decoder (self+cross attention).

Strategy: data-parallel over batch across 8 NeuronCores. Each core runs the
full decoder on its 8-batch-item shard, with activations kept transposed
[C, B_local*T] in SBUF so every projection is a natural lhsT.T @ rhs matmul
with a 512-wide moving dim. Matmul inputs are bf16 (fp32 PSUM accumulate);
residual stream and layernorm statistics stay fp32.
"""

import numpy as np
import ml_dtypes

L, H, C, DH, FF = 6, 8, 512, 64, 2048
B, T = 64, 128
EPS = 1e-5
NCORES = 8
BL = B // NCORES          # batch items per core
TB = BL * T               # 1024 activation columns per core
NC = C // 128             # 4 channel chunks
NF = FF // 128            # 16 ff chunks
NPAIR = H // 2            # head pairs
HD = H * DH               # 512
P = 128

_BF = ml_dtypes.bfloat16

_cache = {}


def _build(n_layers=L, stop=None):
    from contextlib import ExitStack

    import concourse.bass as bass  # noqa: F401
    import concourse.tile as tile
    import concourse.mybir as mybir
    from concourse import bacc

    dt = mybir.dt
    AF = mybir.ActivationFunctionType
    OP = mybir.AluOpType
    f32, bf16, f32r = dt.float32, dt.bfloat16, dt.float32r

    nc = bacc.Bacc("TRN2", target_bir_lowering=False, debug=False, num_devices=NCORES)

    d_xT = nc.dram_tensor("xT", [C, TB], f32, kind="ExternalInput").ap()
    d_xTb = nc.dram_tensor("xTb", [C, TB], bf16, kind="ExternalInput").ap()
    d_eT = nc.dram_tensor("eT", [C, TB], bf16, kind="ExternalInput").ap()
    d_w = {}
    for name in ("wq", "wk", "wv", "cq", "ck", "cv"):
        d_w[name] = nc.dram_tensor(name, [L, P, NC, HD], bf16, kind="ExternalInput").ap()
    d_w1 = nc.dram_tensor("w1", [L, P, NC, FF], bf16, kind="ExternalInput").ap()
    d_w2 = nc.dram_tensor("w2", [L, P, NF, C], bf16, kind="ExternalInput").ap()
    d_bqk = nc.dram_tensor("bqk", [P, L * 16], f32, kind="ExternalInput").ap()
    d_bvb = nc.dram_tensor("bvb", [L, 2, P, HD], f32, kind="ExternalInput").ap()
    d_b1 = nc.dram_tensor("b1", [P, L * NF], f32, kind="ExternalInput").ap()
    d_b2 = nc.dram_tensor("b2", [1, L * C], bf16, kind="ExternalInput").ap()
    d_out = nc.dram_tensor("outT", [C, TB], f32, kind="ExternalOutput").ap()

    with tile.TileContext(nc) as tc:
        with ExitStack() as ctx:
            cpool = ctx.enter_context(tc.tile_pool(name="const", bufs=1))
            apool = ctx.enter_context(tc.tile_pool(name="acts", bufs=1))
            wpool = ctx.enter_context(tc.tile_pool(name="wts", bufs=1))
            tpool = ctx.enter_context(tc.tile_pool(name="tmp", bufs=2))
            ps_pj = ctx.enter_context(tc.tile_pool(name="pj", bufs=3, space="PSUM"))
            ps_sc = ctx.enter_context(tc.tile_pool(name="sc", bufs=3, space="PSUM"))
            ps_ao = ctx.enter_context(tc.tile_pool(name="ao", bufs=2, space="PSUM"))

            # ---- constants ----
            ones_col = cpool.tile([P, 1], bf16, tag="ones_col")
            nc.vector.memset(ones_col, 1.0)

            ones128 = cpool.tile([P, P], f32, tag="ones128")
            nc.vector.memset(ones128, 1.0)
            ones_bf = cpool.tile([1, 512], bf16, tag="ones_bf")
            nc.vector.memset(ones_bf, 1.0)
            eps_t = cpool.tile([P, 1], f32, tag="eps")
            nc.vector.memset(eps_t, EPS)
            zero_t = cpool.tile([P, 1], f32, tag="zero")
            nc.vector.memset(zero_t, 0.0)
            bqk_s = cpool.tile([P, L * 16], f32, tag="bqk")
            nc.sync.dma_start(out=bqk_s, in_=d_bqk)
            b1_s = cpool.tile([P, L * NF], f32, tag="b1")
            nc.sync.dma_start(out=b1_s, in_=d_b1)
            b2_s = cpool.tile([1, L * C], bf16, tag="b2")
            nc.sync.dma_start(out=b2_s, in_=d_b2)

            # ---- persistent activations ----
            xres = [[apool.tile([P, 512], f32, tag=f"xres{k}_{h}", name=f"xres{k}_{h}") for h in range(2)]
                    for k in range(NC)]
            xn = [[apool.tile([P, 512], bf16, tag=f"xn{k}_{h}", name=f"xn{k}_{h}") for h in range(2)]
                  for k in range(NC)]
            eTs = [[apool.tile([P, 512], bf16, tag=f"eT{k}_{h}", name=f"eT{k}_{h}") for h in range(2)]
                   for k in range(NC)]
            for k in range(NC):
                for h in range(2):
                    rs, cs_ = slice(k * P, (k + 1) * P), slice(h * 512, (h + 1) * 512)
                    nc.sync.dma_start(out=xres[k][h], in_=d_xT[rs, cs_])
                    nc.sync.dma_start(out=xn[k][h], in_=d_xTb[rs, cs_])
                    nc.sync.dma_start(out=eTs[k][h], in_=d_eT[rs, cs_])
            qT = [apool.tile([P, TB], bf16, tag=f"qT{k}", name=f"qT{k}") for k in range(NC)]
            kT = [apool.tile([P, TB], bf16, tag=f"kT{k}", name=f"kT{k}") for k in range(NC)]
            kcT = [apool.tile([P, TB], bf16, tag=f"kcT{k}", name=f"kcT{k}") for k in range(NC)]
            vS = [apool.tile([P, HD], bf16, tag=f"v{b}", name=f"v{b}") for b in range(BL)]
            vC = [apool.tile([P, HD], bf16, tag=f"vc{b}", name=f"vc{b}") for b in range(BL)]
            hT = [apool.tile([P, 512], bf16, tag=f"hT{f}", name=f"hT{f}") for f in range(NF)]

            def proj_qk(dst, wt, src, col_of, scale):
                # dst[mc][:, half] = src.T @ wt chunk + bias (per-partition), bf16 out
                for h2 in range(2):
                    for mc in range(NC):
                        pj = ps_pj.tile([P, 512], f32, tag="pj", name="pj")
                        for kc in range(NC):
                            nc.tensor.matmul(pj, wt[:, kc, mc * P:(mc + 1) * P],
                                             src[kc][h2],
                                             start=(kc == 0), stop=(kc == NC - 1))
                        col = col_of(mc)
                        nc.scalar.activation(dst[mc][:, h2 * 512:(h2 + 1) * 512], pj,
                                             AF.Identity,
                                             bias=bqk_s[:, col:col + 1], scale=scale)

            def proj_v(dst, wt, src, bias_tile):
                # dst[b] = src_b @ wt + bias_bcast, layout [S, H*DH], bf16 out
                for b in range(BL):
                    h2, bb = divmod(b, 4)
                    pj = ps_pj.tile([P, 512], f32, tag="pj", name="pj")
                    for kc in range(NC):
                        nc.tensor.matmul(pj, src[kc][h2][:, bb * P:(bb + 1) * P],
                                         wt[:, kc, :],
                                         start=(kc == 0), stop=(kc == NC - 1))
                    nc.vector.tensor_tensor(dst[b], pj, bias_tile, op=OP.add)

            def attention(kTl, vl, ST):
                # scores^T -> exp -> colsum (ones matmul) -> recip -> bcast
                # (outer-product matmul) -> normalize -> out^T += into residual
                for b in range(BL):
                    h2, bb = divmod(b, 4)
                    expT = tpool.tile([P, TB], bf16, tag="expT", name="expT")
                    for p in range(NPAIR):
                        for j in range(2):
                            h = 2 * p + j
                            po = j * 64
                            scp = ps_sc.tile([P, P], f32, tag="sc", name="sc")
                            nc.tensor.matmul(scp,
                                             kTl[p][po:po + 64, b * P:(b + 1) * P],
                                             qT[p][po:po + 64, b * P:(b + 1) * P],
                                             start=True, stop=True,
                                             tile_position=(po, 0))
                            nc.scalar.activation(expT[:, h * P:(h + 1) * P], scp,
                                                 AF.Exp, bias=zero_t, scale=1.0)
                    rc = tpool.tile([1, TB], bf16, tag="rc", name="rc")
                    PTt = tpool.tile([P, TB], bf16, tag="PT", name="PT")
                    for j in range(2):
                        cs = ps_sc.tile([1, 512], f32, tag="sc", name="sc")
                        nc.tensor.matmul(cs, ones_col, expT[:, j * 512:(j + 1) * 512],
                                         start=True, stop=True)
                        with nc.allow_low_precision(reason="softmax recip in bf16"):
                            nc.vector.reciprocal(rc[0:1, j * 512:(j + 1) * 512], cs)
                        dd = ps_pj.tile([P, 512], f32, tag="pj", name="pj")
                        nc.tensor.matmul(dd, ones_bf[0:1, 0:P],
                                         rc[0:1, j * 512:(j + 1) * 512],
                                         start=True, stop=True)
                        nc.vector.tensor_tensor(PTt[:, j * 512:(j + 1) * 512],
                                                expT[:, j * 512:(j + 1) * 512], dd,
                                                op=OP.mult)
                    for p in range(NPAIR):
                        ao = ps_ao.tile([P, P], f32, tag="ao", name="ao")
                        for j in range(2):
                            h = 2 * p + j
                            nc.tensor.matmul(ao[j * 64:(j + 1) * 64, :],
                                             vl[b][:, h * 64:(h + 1) * 64],
                                             PTt[:, h * P:(h + 1) * P],
                                             start=True, stop=True,
                                             tile_position=(0, j * 64))
                        dst = xres[p][h2][:, bb * P:(bb + 1) * P]
                        nc.vector.scalar_tensor_tensor(dst, ao, 0.0, dst,
                                                       op0=OP.add, op1=OP.add,
                                                       accum_out=ST[:, b, p:p + 1])

            def ln_half(ST, h2):
                # stats per batch item over (T, C); sums already accumulated
                # into ST[:, b, 0:4] by the residual-evict ops.
                for bb in range(4):
                    b = h2 * 4 + bb
                    for kc in range(NC):
                        sq = tpool.tile([P, P], bf16, tag="sq", name="sq")
                        nc.scalar.activation(sq, xres[kc][h2][:, bb * P:(bb + 1) * P],
                                             AF.Square, bias=zero_t, scale=1.0,
                                             accum_out=ST[:, b, 4 + kc:5 + kc])
                tot = ps_pj.tile([P, 32], f32, tag="pj", name="pj")
                nc.tensor.matmul(tot, ones128,
                                 ST[:, h2 * 4:(h2 + 1) * 4, :].rearrange("p a b -> p (a b)"),
                                 start=True, stop=True)
                tot3 = tot.rearrange("p (a b) -> p a b", b=8)
                mvs = tpool.tile([P, 8], f32, tag="mvs", name="mvs")
                nc.vector.reduce_sum(mvs[:, 0:4], tot3[:, :, 0:4],
                                     axis=mybir.AxisListType.X)
                nc.vector.reduce_sum(mvs[:, 4:8], tot3[:, :, 4:8],
                                     axis=mybir.AxisListType.X)
                nc.scalar.activation(mvs, mvs, AF.Copy, scale=1.0 / 65536.0)
                m = mvs[:, 0:4]
                var = tpool.tile([P, 4], f32, tag="var", name="var")
                nc.vector.tensor_tensor(var, m, m, op=OP.mult)
                nc.vector.tensor_tensor(var, mvs[:, 4:8], var, op=OP.subtract)
                sd = tpool.tile([P, 4], f32, tag="sd", name="sd")
                nc.scalar.activation(sd, var, AF.Sqrt, bias=eps_t, scale=1.0)
                rr = tpool.tile([P, 4], f32, tag="rr", name="rr")
                nc.vector.reciprocal(rr, sd)
                for bb in range(4):
                    for kc in range(NC):
                        src = xres[kc][h2][:, bb * P:(bb + 1) * P]
                        # bf16 copy first (reads pre-norm values), then the
                        # in-place fp32 normalize (Tile orders the WAR dep)
                        nc.gpsimd.tensor_scalar(xn[kc][h2][:, bb * P:(bb + 1) * P],
                                                src, mvs[:, bb:bb + 1],
                                                rr[:, bb:bb + 1],
                                                op0=OP.subtract, op1=OP.mult)
                        nc.gpsimd.tensor_scalar(src, src, mvs[:, bb:bb + 1],
                                                rr[:, bb:bb + 1],
                                                op0=OP.subtract, op1=OP.mult)

            for l in range(n_layers):
                wts = {}
                for name in ("wq", "wk", "wv", "cq", "ck", "cv"):
                    w = wpool.tile([P, NC, HD], bf16, tag=name, name=name)
                    nc.sync.dma_start(out=w, in_=d_w[name][l])
                    wts[name] = w
                w1s = wpool.tile([P, NC, FF], bf16, tag="w1")
                nc.sync.dma_start(out=w1s, in_=d_w1[l])
                w2s = wpool.tile([P, NF, C], bf16, tag="w2")
                nc.sync.dma_start(out=w2s, in_=d_w2[l])
                bvs = wpool.tile([P, HD], f32, tag="bvs")
                nc.sync.dma_start(out=bvs, in_=d_bvb[l, 0])
                bvc = wpool.tile([P, HD], f32, tag="bvc")
                nc.sync.dma_start(out=bvc, in_=d_bvb[l, 1])

                # --- self attention ---
                ST1 = tpool.tile([P, 8, 8], f32, tag="ST", name="ST")
                proj_qk(qT, wts["wq"], xn, lambda mc: (l * 4 + 0) * 4 + mc, 0.125)
                proj_qk(kT, wts["wk"], xn, lambda mc: (l * 4 + 1) * 4 + mc, 1.0)
                proj_v(vS, wts["wv"], xn, bvs)
                if stop == "qkv":
                    break
                attention(kT, vS, ST1)
                if stop == "sa":
                    break
                # cross K/V from the encoder — independent of LN1, fills PE gaps
                proj_qk(kcT, wts["ck"], eTs, lambda mc: (l * 4 + 3) * 4 + mc, 1.0)
                proj_v(vC, wts["cv"], eTs, bvc)
                ln_half(ST1, 0)
                ln_half(ST1, 1)
                if stop == "ln1":
                    break
                # --- cross attention ---
                ST2 = tpool.tile([P, 8, 8], f32, tag="ST", name="ST")
                proj_qk(qT, wts["cq"], xn, lambda mc: (l * 4 + 2) * 4 + mc, 0.125)
                if stop == "cqkv":
                    break
                attention(kcT, vC, ST2)
                if stop == "ca":
                    break
                ln_half(ST2, 0)
                ln_half(ST2, 1)
                if stop == "ln2":
                    break
                # --- feed-forward ---
                ST3 = tpool.tile([P, 8, 8], f32, tag="ST", name="ST")
                for h2 in range(2):
                    for fc in range(NF):
                        pj = ps_pj.tile([P, 512], f32, tag="pj", name="pj")
                        for kc in range(NC):
                            nc.tensor.matmul(pj, w1s[:, kc, fc * P:(fc + 1) * P],
                                             xn[kc][h2],
                                             start=(kc == 0), stop=(kc == NC - 1))
                        col = l * NF + fc
                        nc.scalar.activation(hT[fc], pj, AF.Relu,
                                             bias=b1_s[:, col:col + 1], scale=1.0)
                    for mc in range(NC):
                        pj = ps_pj.tile([P, 512], f32, tag="pj", name="pj")
                        nc.tensor.matmul(pj, b2_s[0:1, l * C + mc * P:l * C + (mc + 1) * P],
                                         ones_bf[0:1, :], start=True, stop=False)
                        for fc in range(NF):
                            nc.tensor.matmul(pj, w2s[:, fc, mc * P:(mc + 1) * P],
                                             hT[fc],
                                             start=False, stop=(fc == NF - 1))
                        for bb in range(4):
                            b = h2 * 4 + bb
                            dst = xres[mc][h2][:, bb * P:(bb + 1) * P]
                            nc.vector.scalar_tensor_tensor(dst,
                                                           pj[:, bb * P:(bb + 1) * P],
                                                           0.0, dst,
                                                           op0=OP.add, op1=OP.add,
                                                           accum_out=ST3[:, b, mc:mc + 1])
                    ln_half(ST3, h2)

            for k in range(NC):
                for h in range(2):
                    nc.sync.dma_start(
                        out=d_out[k * P:(k + 1) * P, h * 512:(h + 1) * 512],
                        in_=xres[k][h])

    nc.compile()
    return nc


def _prep_shared(inputs):
    """Host-side weight repacking (shared across cores)."""
    def packw(w):  # [L,H,C,DH] -> [L,128,NC,H*DH]  (c = kc*128+p)
        w2 = np.ascontiguousarray(w.transpose(0, 2, 1, 3)).reshape(L, C, HD)
        return np.ascontiguousarray(
            w2.reshape(L, NC, P, HD).transpose(0, 2, 1, 3)).astype(_BF)

    shared = {}
    for nm, key in (("wq", "sa_wq"), ("wk", "sa_wk"), ("wv", "sa_wv"),
                    ("cq", "ca_wq"), ("ck", "ca_wk"), ("cv", "ca_wv")):
        shared[nm] = packw(inputs[key])
    shared["w1"] = np.ascontiguousarray(
        inputs["ff_w1"].reshape(L, NC, P, FF).transpose(0, 2, 1, 3)).astype(_BF)
    shared["w2"] = np.ascontiguousarray(
        inputs["ff_w2"].reshape(L, NF, P, C).transpose(0, 2, 1, 3)).astype(_BF)

    bqk = np.zeros((P, L * 16), np.float32)
    for l in range(L):
        for mi, (bias, s) in enumerate((
                (inputs["sa_bq"][l], 0.125), (inputs["sa_bk"][l], 1.0),
                (inputs["ca_bq"][l], 0.125), (inputs["ca_bk"][l], 1.0))):
            flat = bias.reshape(HD).astype(np.float32) * s
            for mc in range(NC):
                bqk[:, (l * 4 + mi) * 4 + mc] = flat[mc * P:(mc + 1) * P]
    shared["bqk"] = bqk

    bv = np.stack([inputs["sa_bv"].reshape(L, HD),
                   inputs["ca_bv"].reshape(L, HD)], axis=1).astype(np.float32)
    shared["bvb"] = np.ascontiguousarray(
        np.broadcast_to(bv[:, :, None, :], (L, 2, P, HD)))

    b1 = np.zeros((P, L * NF), np.float32)
    for l in range(L):
        for fc in range(NF):
            b1[:, l * NF + fc] = inputs["ff_b1"][l, fc * P:(fc + 1) * P]
    shared["b1"] = b1
    shared["b2"] = inputs["ff_b2"].reshape(1, L * C).astype(_BF)
    return shared


LAST_RESULT = None


def _install_ntff_hook():
    """Register the axon NTFF profile hook that the image's antenv lacks.

    Only used for local benchmarking (KERNEL_TRACE=1); inert otherwise.
    """
    import sys
    import types
    try:
        import antenv
        if getattr(antenv, "axon_hooks", None) is not None:
            return
        from trn_agent_boot.trn_boot import _ntff_profile_via_ctypes
        mod = types.ModuleType("antenv.axon_hooks")
        mod._hook = _ntff_profile_via_ctypes("/opt/axon/libaxon_pjrt.so")

        def get_axon_ntff_profile_hook():
            return mod._hook

        def set_axon_ntff_profile_hook(h):
            mod._hook = h

        mod.get_axon_ntff_profile_hook = get_axon_ntff_profile_hook
        mod.set_axon_ntff_profile_hook = set_axon_ntff_profile_hook
        sys.modules["antenv.axon_hooks"] = mod
        antenv.axon_hooks = mod
    except Exception as e:  # pragma: no cover - profiling is best-effort
        print(f"ntff hook install failed: {e}")


def kernel(**inputs):
    global LAST_RESULT
    import os
    inputs = {k: np.asarray(v) for k, v in inputs.items()}
    if "nc" not in _cache:
        _cache["nc"] = _build()
    nc = _cache["nc"]

    shared = _prep_shared(inputs)
    x = inputs["x"].astype(np.float32)
    enc = inputs["encoder_output"].astype(np.float32)

    in_maps = []
    for core in range(NCORES):
        sl = slice(core * BL, (core + 1) * BL)
        xT = np.ascontiguousarray(x[sl].transpose(2, 0, 1)).reshape(C, TB)
        eT = np.ascontiguousarray(enc[sl].transpose(2, 0, 1)).reshape(C, TB)
        m = dict(shared)
        m["xT"] = xT
        m["xTb"] = xT.astype(_BF)
        m["eT"] = eT.astype(_BF)
        in_maps.append(m)

    trace = bool(int(os.environ.get("KERNEL_TRACE", "0")))
    if trace:
        _install_ntff_hook()
    from concourse.bass_utils import run_bass_kernel_spmd
    res = run_bass_kernel_spmd(nc, in_maps, list(range(NCORES)), trace=trace,
                               trace_cores=[0])
    LAST_RESULT = res

    out = np.empty((B, T, C), np.float32)
    for core in range(NCORES):
        outT = res.results[core]["outT"]  # [C, TB]
        out[core * BL:(core + 1) * BL] = outT.reshape(C, BL, T).transpose(1, 2, 0)
    return out


# revision 26
# speedup vs baseline: 2.4507x; 2.4507x over previous
"""Trainium2 Bass kernel for a 6-layer transformer decoder (self+cross attention).

Strategy: data-parallel over batch across 8 NeuronCores. Each core runs the
full decoder on its 8-batch-item shard, with activations kept transposed
[C, B_local*T] in SBUF so every projection is a natural lhsT.T @ rhs matmul
with a 512-wide moving dim. Matmul inputs are bf16 (fp32 PSUM accumulate);
residual stream and layernorm statistics stay fp32.
"""

import numpy as np
import ml_dtypes

L, H, C, DH, FF = 6, 8, 512, 64, 2048
B, T = 64, 128
EPS = 1e-5
NCORES = 8
BL = B // NCORES          # batch items per core
TB = BL * T               # 1024 activation columns per core
NC = C // 128             # 4 channel chunks
NF = FF // 128            # 16 ff chunks
NPAIR = H // 2            # head pairs
HD = H * DH               # 512
P = 128

_BF = ml_dtypes.bfloat16

_cache = {}


def _build(n_layers=L, stop=None):
    from contextlib import ExitStack

    import concourse.bass as bass  # noqa: F401
    import concourse.tile as tile
    import concourse.mybir as mybir
    from concourse import bacc

    dt = mybir.dt
    AF = mybir.ActivationFunctionType
    OP = mybir.AluOpType
    f32, bf16, f32r = dt.float32, dt.bfloat16, dt.float32r

    nc = bacc.Bacc("TRN2", target_bir_lowering=False, debug=False, num_devices=NCORES)

    d_xT = nc.dram_tensor("xT", [C, TB], f32, kind="ExternalInput").ap()
    d_xTb = nc.dram_tensor("xTb", [C, TB], bf16, kind="ExternalInput").ap()
    d_eT = nc.dram_tensor("eT", [C, TB], bf16, kind="ExternalInput").ap()
    d_w = {}
    for name in ("wq", "wk", "wv", "cq", "ck", "cv"):
        d_w[name] = nc.dram_tensor(name, [L, P, NC, HD], bf16, kind="ExternalInput").ap()
    d_w1 = nc.dram_tensor("w1", [L, P, NC, FF], bf16, kind="ExternalInput").ap()
    d_w2 = nc.dram_tensor("w2", [L, P, NF, C], bf16, kind="ExternalInput").ap()
    d_bqk = nc.dram_tensor("bqk", [P, L * 16], f32, kind="ExternalInput").ap()
    d_bvb = nc.dram_tensor("bvb", [L, 2, P, HD], f32, kind="ExternalInput").ap()
    d_b1 = nc.dram_tensor("b1", [P, L * NF], f32, kind="ExternalInput").ap()
    d_b2 = nc.dram_tensor("b2", [1, L * C], bf16, kind="ExternalInput").ap()
    d_out = nc.dram_tensor("outT", [C, TB], f32, kind="ExternalOutput").ap()

    with tile.TileContext(nc) as tc:
        with ExitStack() as ctx:
            cpool = ctx.enter_context(tc.tile_pool(name="const", bufs=1))
            apool = ctx.enter_context(tc.tile_pool(name="acts", bufs=1))
            wpool = ctx.enter_context(tc.tile_pool(name="wts", bufs=1))
            tpool = ctx.enter_context(tc.tile_pool(name="tmp", bufs=2))
            ps_pj = ctx.enter_context(tc.tile_pool(name="pj", bufs=3, space="PSUM"))
            ps_sc = ctx.enter_context(tc.tile_pool(name="sc", bufs=3, space="PSUM"))
            ps_ao = ctx.enter_context(tc.tile_pool(name="ao", bufs=2, space="PSUM"))

            # ---- constants ----
            ones128b = cpool.tile([P, P], bf16, tag="ones128b")
            nc.vector.memset(ones128b, 1.0)

            ones128 = cpool.tile([P, P], f32, tag="ones128")
            nc.vector.memset(ones128, 1.0)
            ones_bf = cpool.tile([1, 512], bf16, tag="ones_bf")
            nc.vector.memset(ones_bf, 1.0)
            eps_t = cpool.tile([P, 1], f32, tag="eps")
            nc.vector.memset(eps_t, EPS)
            zero_t = cpool.tile([P, 1], f32, tag="zero")
            nc.vector.memset(zero_t, 0.0)
            bqk_s = cpool.tile([P, L * 16], f32, tag="bqk")
            nc.sync.dma_start(out=bqk_s, in_=d_bqk)
            b1_s = cpool.tile([P, L * NF], f32, tag="b1")
            nc.sync.dma_start(out=b1_s, in_=d_b1)
            b2_s = cpool.tile([1, L * C], bf16, tag="b2")
            nc.sync.dma_start(out=b2_s, in_=d_b2)

            # ---- persistent activations ----
            xres = [[apool.tile([P, 512], f32, tag=f"xres{k}_{h}", name=f"xres{k}_{h}") for h in range(2)]
                    for k in range(NC)]
            xn = [[apool.tile([P, 512], bf16, tag=f"xn{k}_{h}", name=f"xn{k}_{h}") for h in range(2)]
                  for k in range(NC)]
            eTs = [[apool.tile([P, 512], bf16, tag=f"eT{k}_{h}", name=f"eT{k}_{h}") for h in range(2)]
                   for k in range(NC)]
            for k in range(NC):
                for h in range(2):
                    rs, cs_ = slice(k * P, (k + 1) * P), slice(h * 512, (h + 1) * 512)
                    nc.sync.dma_start(out=xres[k][h], in_=d_xT[rs, cs_])
                    nc.sync.dma_start(out=xn[k][h], in_=d_xTb[rs, cs_])
                    nc.sync.dma_start(out=eTs[k][h], in_=d_eT[rs, cs_])
            qT = [apool.tile([P, TB], bf16, tag=f"qT{k}", name=f"qT{k}") for k in range(NC)]
            kT = [apool.tile([P, TB], bf16, tag=f"kT{k}", name=f"kT{k}") for k in range(NC)]
            kcT = [apool.tile([P, TB], bf16, tag=f"kcT{k}", name=f"kcT{k}") for k in range(NC)]
            vS = [apool.tile([P, HD], bf16, tag=f"v{b}", name=f"v{b}") for b in range(BL)]
            vC = [apool.tile([P, HD], bf16, tag=f"vc{b}", name=f"vc{b}") for b in range(BL)]
            hT = [apool.tile([P, 512], bf16, tag=f"hT{f}", name=f"hT{f}") for f in range(NF)]

            def proj_qk(dst, wt, src, col_of, scale):
                # dst[mc][:, half] = src.T @ wt chunk + bias (per-partition), bf16 out
                for h2 in range(2):
                    for mc in range(NC):
                        pj = ps_pj.tile([P, 512], f32, tag="pj", name="pj")
                        for kc in range(NC):
                            nc.tensor.matmul(pj, wt[:, kc, mc * P:(mc + 1) * P],
                                             src[kc][h2],
                                             start=(kc == 0), stop=(kc == NC - 1))
                        col = col_of(mc)
                        nc.scalar.activation(dst[mc][:, h2 * 512:(h2 + 1) * 512], pj,
                                             AF.Identity,
                                             bias=bqk_s[:, col:col + 1], scale=scale)

            def proj_v(dst, wt, src, bias_tile):
                # dst[b] = src_b @ wt + bias_bcast, layout [S, H*DH], bf16 out
                for b in range(BL):
                    h2, bb = divmod(b, 4)
                    pj = ps_pj.tile([P, 512], f32, tag="pj", name="pj")
                    for kc in range(NC):
                        nc.tensor.matmul(pj, src[kc][h2][:, bb * P:(bb + 1) * P],
                                         wt[:, kc, :],
                                         start=(kc == 0), stop=(kc == NC - 1))
                    nc.vector.tensor_tensor(dst[b], pj, bias_tile, op=OP.add)

            def attention(kTl, vl, ST):
                # scores^T (2-head row-packed, even/odd banks) -> exp
                # ([128,512] per bank) -> broadcast colsums via a full ones
                # matmul (every output row = column sum) -> approx recip ->
                # normalize expT in place -> out^T += into residual
                # expT column layout: h at (h%2)*512 + (h//2)*128
                for b in range(BL):
                    h2, bb = divmod(b, 4)
                    expT = tpool.tile([P, TB], bf16, tag="expT", name="expT")
                    sce = ps_sc.tile([P, 512], f32, tag="sc", name="sc")
                    sco = ps_sc.tile([P, 512], f32, tag="sc", name="sc")
                    for p in range(NPAIR):
                        nc.tensor.matmul(sce[:, p * P:(p + 1) * P],
                                         kTl[p][0:64, b * P:(b + 1) * P],
                                         qT[p][0:64, b * P:(b + 1) * P],
                                         start=True, stop=True,
                                         tile_position=(0, 0))
                        nc.tensor.matmul(sco[:, p * P:(p + 1) * P],
                                         kTl[p][64:128, b * P:(b + 1) * P],
                                         qT[p][64:128, b * P:(b + 1) * P],
                                         start=True, stop=True,
                                         tile_position=(64, 0))
                    nc.scalar.activation(expT[:, 0:512], sce, AF.Exp,
                                         bias=zero_t, scale=1.0)
                    nc.scalar.activation(expT[:, 512:1024], sco, AF.Exp,
                                         bias=zero_t, scale=1.0)
                    for j in range(2):
                        sl = slice(j * 512, (j + 1) * 512)
                        dsum = ps_pj.tile([P, 512], f32, tag="pj", name="pj")
                        nc.tensor.matmul(dsum, ones128b, expT[:, sl],
                                         start=True, stop=True)
                        ddr = tpool.tile([P, 512], f32, tag="ddr", name="ddr")
                        # ACT-engine reciprocal (~1.2e-5 rel err measured on
                        # hw for this value range); bass's wrapper refuses
                        # Reciprocal so emit the instruction directly.
                        nc.scalar.add_instruction(mybir.InstActivation(
                            name=nc.get_next_instruction_name(),
                            func=AF.Reciprocal,
                            ins=[nc.scalar.lower_ap(dsum),
                                 mybir.ImmediateValue(dtype=f32, value=0.0),
                                 mybir.ImmediateValue(dtype=f32, value=1.0),
                                 mybir.ImmediateValue(dtype=f32, value=0.0)],
                            outs=[nc.scalar.lower_ap(ddr)],
                        ))
                        nc.vector.tensor_tensor(expT[:, sl], expT[:, sl], ddr,
                                                op=OP.mult)
                    for p in range(NPAIR):
                        ao = ps_ao.tile([P, P], f32, tag="ao", name="ao")
                        for j in range(2):
                            h = 2 * p + j
                            pos = (h % 2) * 512 + (h // 2) * P
                            nc.tensor.matmul(ao[j * 64:(j + 1) * 64, :],
                                             vl[b][:, h * 64:(h + 1) * 64],
                                             expT[:, pos:pos + P],
                                             start=True, stop=True,
                                             tile_position=(0, j * 64))
                        dst = xres[p][h2][:, bb * P:(bb + 1) * P]
                        nc.vector.scalar_tensor_tensor(dst, ao, 0.0, dst,
                                                       op0=OP.add, op1=OP.add,
                                                       accum_out=ST[:, b, p:p + 1])

            def ln_half(ST, h2):
                # stats per batch item over (T, C); sums already accumulated
                # into ST[:, b, 0:4] by the residual-evict ops.
                for bb in range(4):
                    b = h2 * 4 + bb
                    for kc in range(NC):
                        sq = tpool.tile([P, P], bf16, tag="sq", name="sq")
                        src = xres[kc][h2][:, bb * P:(bb + 1) * P]
                        nc.vector.scalar_tensor_tensor(
                            sq, src, 1.0, src, op0=OP.mult, op1=OP.mult,
                            accum_out=ST[:, b, 4 + kc:5 + kc])
                tot = ps_pj.tile([P, 32], f32, tag="pj", name="pj")
                nc.tensor.matmul(tot, ones128,
                                 ST[:, h2 * 4:(h2 + 1) * 4, :].rearrange("p a b -> p (a b)"),
                                 start=True, stop=True)
                tot3 = tot.rearrange("p (a b) -> p a b", b=8)
                mvs = tpool.tile([P, 8], f32, tag="mvs", name="mvs")
                nc.vector.reduce_sum(mvs[:, 0:4], tot3[:, :, 0:4],
                                     axis=mybir.AxisListType.X)
                nc.vector.reduce_sum(mvs[:, 4:8], tot3[:, :, 4:8],
                                     axis=mybir.AxisListType.X)
                nc.scalar.activation(mvs, mvs, AF.Copy, scale=1.0 / 65536.0)
                m = mvs[:, 0:4]
                var = tpool.tile([P, 4], f32, tag="var", name="var")
                nc.vector.tensor_tensor(var, m, m, op=OP.mult)
                nc.vector.tensor_tensor(var, mvs[:, 4:8], var, op=OP.subtract)
                sd = tpool.tile([P, 4], f32, tag="sd", name="sd")
                nc.scalar.activation(sd, var, AF.Sqrt, bias=eps_t, scale=1.0)
                rr = tpool.tile([P, 4], f32, tag="rr", name="rr")
                nc.vector.reciprocal(rr, sd)
                nb = tpool.tile([P, 4], f32, tag="nb", name="nb")
                nc.vector.tensor_scalar(nb, mvs[:, 0:4], -1.0, None, op0=OP.mult)
                nc.vector.tensor_tensor(nb, nb, rr, op=OP.mult)
                for bb in range(4):
                    for kc in range(NC):
                        src = xres[kc][h2][:, bb * P:(bb + 1) * P]
                        # bf16 x*r + (-m*r) on ACT (reads pre-norm values),
                        # then in-place fp32 normalize on DVE (Tile orders WAR)
                        nc.scalar.activation(xn[kc][h2][:, bb * P:(bb + 1) * P],
                                             src, AF.Identity,
                                             bias=nb[:, bb:bb + 1],
                                             scale=rr[:, bb:bb + 1])
                        nc.vector.tensor_scalar(src, src, mvs[:, bb:bb + 1],
                                                rr[:, bb:bb + 1],
                                                op0=OP.subtract, op1=OP.mult)

            for l in range(n_layers):
                wts = {}
                for name in ("wq", "wk", "wv", "cq", "ck", "cv"):
                    w = wpool.tile([P, NC, HD], bf16, tag=name, name=name)
                    nc.sync.dma_start(out=w, in_=d_w[name][l])
                    wts[name] = w
                w1s = wpool.tile([P, NC, FF], bf16, tag="w1")
                nc.sync.dma_start(out=w1s, in_=d_w1[l])
                w2s = wpool.tile([P, NF, C], bf16, tag="w2")
                nc.sync.dma_start(out=w2s, in_=d_w2[l])
                bvs = wpool.tile([P, HD], f32, tag="bvs")
                nc.sync.dma_start(out=bvs, in_=d_bvb[l, 0])
                bvc = wpool.tile([P, HD], f32, tag="bvc")
                nc.sync.dma_start(out=bvc, in_=d_bvb[l, 1])

                # --- self attention ---
                ST1 = tpool.tile([P, 8, 8], f32, tag="ST", name="ST")
                proj_qk(qT, wts["wq"], xn, lambda mc: (l * 4 + 0) * 4 + mc, 0.125)
                proj_qk(kT, wts["wk"], xn, lambda mc: (l * 4 + 1) * 4 + mc, 1.0)
                proj_v(vS, wts["wv"], xn, bvs)
                if stop == "qkv":
                    break
                attention(kT, vS, ST1)
                if stop == "sa":
                    break
                # cross K/V from the encoder — independent of LN1, fills PE gaps
                proj_qk(kcT, wts["ck"], eTs, lambda mc: (l * 4 + 3) * 4 + mc, 1.0)
                proj_v(vC, wts["cv"], eTs, bvc)
                ln_half(ST1, 0)
                ln_half(ST1, 1)
                if stop == "ln1":
                    break
                # --- cross attention ---
                ST2 = tpool.tile([P, 8, 8], f32, tag="ST", name="ST")
                proj_qk(qT, wts["cq"], xn, lambda mc: (l * 4 + 2) * 4 + mc, 0.125)
                if stop == "cqkv":
                    break
                attention(kcT, vC, ST2)
                if stop == "ca":
                    break
                ln_half(ST2, 0)
                ln_half(ST2, 1)
                if stop == "ln2":
                    break
                # --- feed-forward ---
                ST3 = tpool.tile([P, 8, 8], f32, tag="ST", name="ST")
                for h2 in range(2):
                    for fc in range(NF):
                        pj = ps_pj.tile([P, 512], f32, tag="pj", name="pj")
                        for kc in range(NC):
                            nc.tensor.matmul(pj, w1s[:, kc, fc * P:(fc + 1) * P],
                                             xn[kc][h2],
                                             start=(kc == 0), stop=(kc == NC - 1))
                        col = l * NF + fc
                        nc.scalar.activation(hT[fc], pj, AF.Relu,
                                             bias=b1_s[:, col:col + 1], scale=1.0)
                    for mc in range(NC):
                        pj = ps_pj.tile([P, 512], f32, tag="pj", name="pj")
                        nc.tensor.matmul(pj, b2_s[0:1, l * C + mc * P:l * C + (mc + 1) * P],
                                         ones_bf[0:1, :], start=True, stop=False)
                        for fc in range(NF):
                            nc.tensor.matmul(pj, w2s[:, fc, mc * P:(mc + 1) * P],
                                             hT[fc],
                                             start=False, stop=(fc == NF - 1))
                        for bb in range(4):
                            b = h2 * 4 + bb
                            dst = xres[mc][h2][:, bb * P:(bb + 1) * P]
                            nc.vector.scalar_tensor_tensor(dst,
                                                           pj[:, bb * P:(bb + 1) * P],
                                                           0.0, dst,
                                                           op0=OP.add, op1=OP.add,
                                                           accum_out=ST3[:, b, mc:mc + 1])
                    ln_half(ST3, h2)

            for k in range(NC):
                for h in range(2):
                    nc.sync.dma_start(
                        out=d_out[k * P:(k + 1) * P, h * 512:(h + 1) * 512],
                        in_=xres[k][h])

    nc.compile()
    return nc


def _prep_shared(inputs):
    """Host-side weight repacking (shared across cores)."""
    def packw(w):  # [L,H,C,DH] -> [L,128,NC,H*DH]  (c = kc*128+p)
        w2 = np.ascontiguousarray(w.transpose(0, 2, 1, 3)).reshape(L, C, HD)
        return np.ascontiguousarray(
            w2.reshape(L, NC, P, HD).transpose(0, 2, 1, 3)).astype(_BF)

    shared = {}
    for nm, key in (("wq", "sa_wq"), ("wk", "sa_wk"), ("wv", "sa_wv"),
                    ("cq", "ca_wq"), ("ck", "ca_wk"), ("cv", "ca_wv")):
        shared[nm] = packw(inputs[key])
    shared["w1"] = np.ascontiguousarray(
        inputs["ff_w1"].reshape(L, NC, P, FF).transpose(0, 2, 1, 3)).astype(_BF)
    shared["w2"] = np.ascontiguousarray(
        inputs["ff_w2"].reshape(L, NF, P, C).transpose(0, 2, 1, 3)).astype(_BF)

    bqk = np.zeros((P, L * 16), np.float32)
    for l in range(L):
        for mi, (bias, s) in enumerate((
                (inputs["sa_bq"][l], 0.125), (inputs["sa_bk"][l], 1.0),
                (inputs["ca_bq"][l], 0.125), (inputs["ca_bk"][l], 1.0))):
            flat = bias.reshape(HD).astype(np.float32) * s
            for mc in range(NC):
                bqk[:, (l * 4 + mi) * 4 + mc] = flat[mc * P:(mc + 1) * P]
    shared["bqk"] = bqk

    bv = np.stack([inputs["sa_bv"].reshape(L, HD),
                   inputs["ca_bv"].reshape(L, HD)], axis=1).astype(np.float32)
    shared["bvb"] = np.ascontiguousarray(
        np.broadcast_to(bv[:, :, None, :], (L, 2, P, HD)))

    b1 = np.zeros((P, L * NF), np.float32)
    for l in range(L):
        for fc in range(NF):
            b1[:, l * NF + fc] = inputs["ff_b1"][l, fc * P:(fc + 1) * P]
    shared["b1"] = b1
    shared["b2"] = inputs["ff_b2"].reshape(1, L * C).astype(_BF)
    return shared


LAST_RESULT = None


def _install_ntff_hook():
    """Register the axon NTFF profile hook that the image's antenv lacks.

    Only used for local benchmarking (KERNEL_TRACE=1); inert otherwise.
    """
    import sys
    import types
    try:
        import antenv
        if getattr(antenv, "axon_hooks", None) is not None:
            return
        from trn_agent_boot.trn_boot import _ntff_profile_via_ctypes
        mod = types.ModuleType("antenv.axon_hooks")
        mod._hook = _ntff_profile_via_ctypes("/opt/axon/libaxon_pjrt.so")

        def get_axon_ntff_profile_hook():
            return mod._hook

        def set_axon_ntff_profile_hook(h):
            mod._hook = h

        mod.get_axon_ntff_profile_hook = get_axon_ntff_profile_hook
        mod.set_axon_ntff_profile_hook = set_axon_ntff_profile_hook
        sys.modules["antenv.axon_hooks"] = mod
        antenv.axon_hooks = mod
    except Exception as e:  # pragma: no cover - profiling is best-effort
        print(f"ntff hook install failed: {e}")


def kernel(**inputs):
    global LAST_RESULT
    import os
    inputs = {k: np.asarray(v) for k, v in inputs.items()}
    if "nc" not in _cache:
        _cache["nc"] = _build()
    nc = _cache["nc"]

    shared = _prep_shared(inputs)
    x = inputs["x"].astype(np.float32)
    enc = inputs["encoder_output"].astype(np.float32)

    in_maps = []
    for core in range(NCORES):
        sl = slice(core * BL, (core + 1) * BL)
        xT = np.ascontiguousarray(x[sl].transpose(2, 0, 1)).reshape(C, TB)
        eT = np.ascontiguousarray(enc[sl].transpose(2, 0, 1)).reshape(C, TB)
        m = dict(shared)
        m["xT"] = xT
        m["xTb"] = xT.astype(_BF)
        m["eT"] = eT.astype(_BF)
        in_maps.append(m)

    trace = bool(int(os.environ.get("KERNEL_TRACE", "0")))
    if trace:
        _install_ntff_hook()
    from concourse.bass_utils import run_bass_kernel_spmd
    res = run_bass_kernel_spmd(nc, in_maps, list(range(NCORES)), trace=trace,
                               trace_cores=[0])
    LAST_RESULT = res

    out = np.empty((B, T, C), np.float32)
    for core in range(NCORES):
        outT = res.results[core]["outT"]  # [C, TB]
        out[core * BL:(core + 1) * BL] = outT.reshape(C, BL, T).transpose(1, 2, 0)
    return out


# revision 28
# speedup vs baseline: 3.0907x; 1.2611x over previous
"""Trainium2 Bass kernel for a 6-layer transformer decoder (self+cross attention).

Strategy: data-parallel over batch across 8 NeuronCores. Each core runs the
full decoder on its 8-batch-item shard, with activations kept transposed
[C, B_local*T] in SBUF so every projection is a natural lhsT.T @ rhs matmul
with a 512-wide moving dim. Matmul inputs are bf16 (fp32 PSUM accumulate);
residual stream and layernorm statistics stay fp32.
"""

import numpy as np
import ml_dtypes

L, H, C, DH, FF = 6, 8, 512, 64, 2048
B, T = 64, 128
EPS = 1e-5
NCORES = 8
BL = B // NCORES          # batch items per core
TB = BL * T               # 1024 activation columns per core
NC = C // 128             # 4 channel chunks
NF = FF // 128            # 16 ff chunks
NPAIR = H // 2            # head pairs
HD = H * DH               # 512
P = 128
NORM = 1.0 / (T * C)      # layernorm 1/N, folded into the stats matmul

_BF = ml_dtypes.bfloat16

_cache = {}


def _build():
    from contextlib import ExitStack

    import concourse.bass as bass  # noqa: F401
    import concourse.tile as tile
    import concourse.mybir as mybir
    from concourse import bacc

    dt = mybir.dt
    AF = mybir.ActivationFunctionType
    OP = mybir.AluOpType
    f32, bf16 = dt.float32, dt.bfloat16

    nc = bacc.Bacc("TRN2", target_bir_lowering=False, debug=False, num_devices=NCORES)

    d_xT = nc.dram_tensor("xT", [C, TB], f32, kind="ExternalInput").ap()
    d_xTb = nc.dram_tensor("xTb", [C, TB], bf16, kind="ExternalInput").ap()
    d_eT = nc.dram_tensor("eT", [C, TB], bf16, kind="ExternalInput").ap()
    d_w = {}
    for name in ("wq", "wk", "wv", "cq", "ck", "cv"):
        d_w[name] = nc.dram_tensor(name, [L, P, NC, HD], bf16, kind="ExternalInput").ap()
    d_w1 = nc.dram_tensor("w1", [L, P, NC, FF], bf16, kind="ExternalInput").ap()
    d_w2 = nc.dram_tensor("w2", [L, P, NF, C], bf16, kind="ExternalInput").ap()
    d_bqk = nc.dram_tensor("bqk", [P, L * 16], f32, kind="ExternalInput").ap()
    d_bvb = nc.dram_tensor("bvb", [L, 2, P, HD], f32, kind="ExternalInput").ap()
    d_b1 = nc.dram_tensor("b1", [P, L * NF], f32, kind="ExternalInput").ap()
    d_b2 = nc.dram_tensor("b2", [1, L * C], bf16, kind="ExternalInput").ap()
    d_out = nc.dram_tensor("outT", [C, TB], f32, kind="ExternalOutput").ap()

    def act_recip(out, in_):
        # ACT-engine reciprocal (~1e-5 rel err measured on hw for this value
        # range); bass's wrapper refuses Reciprocal so emit directly.
        nc.scalar.add_instruction(mybir.InstActivation(
            name=nc.get_next_instruction_name(),
            func=AF.Reciprocal,
            ins=[nc.scalar.lower_ap(in_),
                 mybir.ImmediateValue(dtype=f32, value=0.0),
                 mybir.ImmediateValue(dtype=f32, value=1.0),
                 mybir.ImmediateValue(dtype=f32, value=0.0)],
            outs=[nc.scalar.lower_ap(out)],
        ))

    with tile.TileContext(nc) as tc:
        with ExitStack() as ctx:
            cpool = ctx.enter_context(tc.tile_pool(name="const", bufs=1))
            apool = ctx.enter_context(tc.tile_pool(name="acts", bufs=1))
            wpool = ctx.enter_context(tc.tile_pool(name="wts", bufs=1))
            tpool = ctx.enter_context(tc.tile_pool(name="tmp", bufs=2))
            ps_pj = ctx.enter_context(tc.tile_pool(name="pj", bufs=2, space="PSUM"))
            ps_sc = ctx.enter_context(tc.tile_pool(name="sc", bufs=4, space="PSUM"))
            ps_ao = ctx.enter_context(tc.tile_pool(name="ao", bufs=2, space="PSUM"))

            # ---- constants ----
            ones128b = cpool.tile([P, P], bf16, tag="ones128b")
            nc.vector.memset(ones128b, 1.0)
            onesN = cpool.tile([P, P], f32, tag="onesN")
            nc.vector.memset(onesN, NORM)      # ones/65536 for LN stats matmul
            ones_bf = cpool.tile([1, 512], bf16, tag="ones_bf")
            nc.vector.memset(ones_bf, 1.0)
            eps_t = cpool.tile([P, 1], f32, tag="eps")
            nc.vector.memset(eps_t, EPS)
            zero_t = cpool.tile([P, 1], f32, tag="zero")
            nc.vector.memset(zero_t, 0.0)
            bqk_s = cpool.tile([P, L * 16], f32, tag="bqk")
            nc.sync.dma_start(out=bqk_s, in_=d_bqk)
            b1_s = cpool.tile([P, L * NF], f32, tag="b1")
            nc.sync.dma_start(out=b1_s, in_=d_b1)
            b2_s = cpool.tile([1, L * C], bf16, tag="b2")
            nc.sync.dma_start(out=b2_s, in_=d_b2)

            # ---- persistent activations (kc-major merged tiles per half) ----
            xres = [apool.tile([P, NC * 512], f32, tag=f"xres{h}", name=f"xres{h}")
                    for h in range(2)]
            xn = [apool.tile([P, NC * 512], bf16, tag=f"xn{h}", name=f"xn{h}")
                  for h in range(2)]
            eTs = [apool.tile([P, NC * 512], bf16, tag=f"eT{h}", name=f"eT{h}")
                   for h in range(2)]
            for k in range(NC):
                for h in range(2):
                    rs = slice(k * P, (k + 1) * P)
                    cs_ = slice(h * 512, (h + 1) * 512)
                    ts_ = slice(k * 512, (k + 1) * 512)
                    nc.sync.dma_start(out=xres[h][:, ts_], in_=d_xT[rs, cs_])
                    nc.sync.dma_start(out=xn[h][:, ts_], in_=d_xTb[rs, cs_])
                    nc.sync.dma_start(out=eTs[h][:, ts_], in_=d_eT[rs, cs_])

            def xsl(h2, kc, bb=None):
                if bb is None:
                    return slice(kc * 512, (kc + 1) * 512)
                return slice(kc * 512 + bb * P, kc * 512 + (bb + 1) * P)

            qT = [apool.tile([P, TB], bf16, tag=f"qT{k}", name=f"qT{k}") for k in range(NC)]
            kT = [apool.tile([P, TB], bf16, tag=f"kT{k}", name=f"kT{k}") for k in range(NC)]
            kcT = [apool.tile([P, TB], bf16, tag=f"kcT{k}", name=f"kcT{k}") for k in range(NC)]
            vS = [apool.tile([P, HD], bf16, tag=f"v{b}", name=f"v{b}") for b in range(BL)]
            vC = [apool.tile([P, HD], bf16, tag=f"vc{b}", name=f"vc{b}") for b in range(BL)]
            hT = [apool.tile([P, 512], bf16, tag=f"hT{f}", name=f"hT{f}") for f in range(NF)]

            def proj_qk(dst, wt, src, col_of, scale, halves=(0, 1)):
                for h2 in halves:
                    for mc in range(NC):
                        pj = ps_pj.tile([P, 512], f32, tag="pj", name="pj")
                        for kc in range(NC):
                            nc.tensor.matmul(pj, wt[:, kc, mc * P:(mc + 1) * P],
                                             src[h2][:, xsl(h2, kc)],
                                             start=(kc == 0), stop=(kc == NC - 1))
                        col = col_of(mc)
                        nc.scalar.activation(dst[mc][:, h2 * 512:(h2 + 1) * 512], pj,
                                             AF.Identity,
                                             bias=bqk_s[:, col:col + 1], scale=scale)

            def proj_v(dst, wt, src, bias_tile, bs):
                for b in bs:
                    h2, bb = divmod(b, 4)
                    pj = ps_pj.tile([P, 512], f32, tag="pj", name="pj")
                    for kc in range(NC):
                        nc.tensor.matmul(pj, src[h2][:, xsl(h2, kc, bb)],
                                         wt[:, kc, :],
                                         start=(kc == 0), stop=(kc == NC - 1))
                    nc.vector.tensor_tensor(dst[b], pj, bias_tile, op=OP.add)

            def attention_half(kTl, vl, ST, h2, filler=None):
                # Per half (4 batch items), in 2-batch groups, with ACT ops
                # batched by function to avoid activation-table reloads:
                # all scores -> all exp -> all colsum-bcast matmuls -> all
                # reciprocals -> all normalizes -> all attn-out + evict.
                expTs = {}
                pending = []
                for bb in range(4):
                    b = h2 * 4 + bb
                    expT = tpool.tile([P, TB], bf16, tag="expT", name="expT",
                                      bufs=4)
                    sce = ps_sc.tile([P, 512], f32, tag="sc", name="sc")
                    sco = ps_sc.tile([P, 512], f32, tag="sc", name="sc")
                    for p in range(NPAIR):
                        nc.tensor.matmul(sce[:, p * P:(p + 1) * P],
                                         kTl[p][0:64, b * P:(b + 1) * P],
                                         qT[p][0:64, b * P:(b + 1) * P],
                                         start=True, stop=True,
                                         tile_position=(0, 0))
                        nc.tensor.matmul(sco[:, p * P:(p + 1) * P],
                                         kTl[p][64:128, b * P:(b + 1) * P],
                                         qT[p][64:128, b * P:(b + 1) * P],
                                         start=True, stop=True,
                                         tile_position=(64, 0))
                    pending.append((bb, expT, sce, sco))
                    expTs[bb] = expT
                    if len(pending) == 2 or bb == 3:
                        for _bb, _e, _sce, _sco in pending:
                            nc.scalar.activation(_e[:, 0:512], _sce, AF.Exp,
                                                 bias=zero_t, scale=1.0)
                            nc.scalar.activation(_e[:, 512:1024], _sco, AF.Exp,
                                                 bias=zero_t, scale=1.0)
                        pending = []
                # colsum broadcast matmuls, then all reciprocals, then mults
                ddrs = {}
                for bb in range(4):
                    for j in range(2):
                        sl = slice(j * 512, (j + 1) * 512)
                        dsum = ps_sc.tile([P, 512], f32, tag="sc", name="sc",
                                          bufs=4)
                        nc.tensor.matmul(dsum, ones128b, expTs[bb][:, sl],
                                         start=True, stop=True)
                        ddr = tpool.tile([P, 512], bf16, tag="ddr", name="ddr",
                                         bufs=6)
                        act_recip(ddr, dsum)
                        ddrs[(bb, j)] = ddr
                if filler is not None:
                    filler()
                for bb in range(4):
                    for j in range(2):
                        sl = slice(j * 512, (j + 1) * 512)
                        nc.vector.tensor_tensor(expTs[bb][:, sl],
                                                expTs[bb][:, sl],
                                                ddrs[(bb, j)], op=OP.mult)
                for bb in range(4):
                    b = h2 * 4 + bb
                    for p in range(NPAIR):
                        ao = ps_ao.tile([P, P], f32, tag="ao", name="ao")
                        for j in range(2):
                            h = 2 * p + j
                            pos = (h % 2) * 512 + (h // 2) * P
                            nc.tensor.matmul(ao[j * 64:(j + 1) * 64, :],
                                             vl[b][:, h * 64:(h + 1) * 64],
                                             expTs[bb][:, pos:pos + P],
                                             start=True, stop=True,
                                             tile_position=(0, j * 64))
                        dst = xres[h2][:, xsl(h2, p, bb)]
                        nc.vector.scalar_tensor_tensor(dst, ao, 0.0, dst,
                                                       op0=OP.add, op1=OP.add,
                                                       accum_out=ST[:, b, p:p + 1])

            def ln_half(ST, h2, last=False):
                # stats per batch item over (T, C); per-chunk sums were
                # accumulated into ST[:, b, 0:4] by the residual-evict ops;
                # a single fused square+reduce per item fills ST[:, b, 4].
                x3 = xres[h2].rearrange("p (k n) -> p k n", k=NC)
                for bb in range(4):
                    b = h2 * 4 + bb
                    sq = tpool.tile([P, 512], bf16, tag="sq", name="sq")
                    src = x3[:, :, bb * P:(bb + 1) * P]
                    nc.vector.scalar_tensor_tensor(
                        sq.rearrange("p (k n) -> p k n", k=NC), src, 1.0, src,
                        op0=OP.mult, op1=OP.mult,
                        accum_out=ST[:, b, 4:5])
                tot = ps_pj.tile([P, 20], f32, tag="pj", name="pj")
                nc.tensor.matmul(tot, onesN,
                                 ST[:, h2 * 4:(h2 + 1) * 4, :].rearrange("p a b -> p (a b)"),
                                 start=True, stop=True)
                tot3 = tot.rearrange("p (a b) -> p a b", b=5)
                mm_ = tpool.tile([P, 4], f32, tag="mm_", name="mm_")
                nc.vector.reduce_sum(mm_, tot3[:, :, 0:4], axis=mybir.AxisListType.X)
                var = tpool.tile([P, 4], f32, tag="var", name="var")
                nc.vector.tensor_tensor(var, mm_, mm_, op=OP.mult)
                nc.vector.tensor_tensor(var, tot3[:, :, 4], var, op=OP.subtract)
                sd = tpool.tile([P, 4], f32, tag="sd", name="sd")
                nc.scalar.activation(sd, var, AF.Sqrt, bias=eps_t, scale=1.0)
                rr = tpool.tile([P, 4], f32, tag="rr", name="rr")
                nc.vector.reciprocal(rr, sd)
                for bb in range(4):
                    src = x3[:, :, bb * P:(bb + 1) * P]
                    nc.vector.tensor_scalar(src, src, mm_[:, bb:bb + 1],
                                            rr[:, bb:bb + 1],
                                            op0=OP.subtract, op1=OP.mult)
                if not last:
                    for j in range(2):
                        sl = slice(j * TB, (j + 1) * TB)
                        nc.vector.tensor_copy(out=xn[h2][:, sl],
                                              in_=xres[h2][:, sl])

            for l in range(L):
                wts = {}
                for name in ("wq", "wk", "wv", "cq", "ck", "cv"):
                    w = wpool.tile([P, NC, HD], bf16, tag=name, name=name)
                    nc.sync.dma_start(out=w, in_=d_w[name][l])
                    wts[name] = w
                w1s = wpool.tile([P, NC, FF], bf16, tag="w1", name="w1")
                nc.sync.dma_start(out=w1s, in_=d_w1[l])
                w2s = wpool.tile([P, NF, C], bf16, tag="w2", name="w2")
                nc.sync.dma_start(out=w2s, in_=d_w2[l])
                bvs = wpool.tile([P, HD], f32, tag="bvs", name="bvs")
                nc.sync.dma_start(out=bvs, in_=d_bvb[l, 0])
                bvc = wpool.tile([P, HD], f32, tag="bvc", name="bvc")
                nc.sync.dma_start(out=bvc, in_=d_bvb[l, 1])

                # --- self attention (cross K/V emitted as PE filler) ---
                ST1 = tpool.tile([P, 8, 5], f32, tag="ST", name="ST")
                proj_qk(qT, wts["wq"], xn, lambda mc: (l * 4 + 0) * 4 + mc, 0.125)
                proj_qk(kT, wts["wk"], xn, lambda mc: (l * 4 + 1) * 4 + mc, 1.0)
                proj_v(vS, wts["wv"], xn, bvs, range(BL))

                def filler0():
                    proj_qk(kcT, wts["ck"], eTs,
                            lambda mc: (l * 4 + 3) * 4 + mc, 1.0, halves=(0,))
                    proj_v(vC, wts["cv"], eTs, bvc, range(0, 4))

                def filler1():
                    proj_qk(kcT, wts["ck"], eTs,
                            lambda mc: (l * 4 + 3) * 4 + mc, 1.0, halves=(1,))
                    proj_v(vC, wts["cv"], eTs, bvc, range(4, 8))

                attention_half(kT, vS, ST1, 0, filler=filler0)
                attention_half(kT, vS, ST1, 1, filler=filler1)
                ln_half(ST1, 0)
                ln_half(ST1, 1)
                # --- cross attention ---
                ST2 = tpool.tile([P, 8, 5], f32, tag="ST", name="ST")
                proj_qk(qT, wts["cq"], xn, lambda mc: (l * 4 + 2) * 4 + mc, 0.125)
                attention_half(kcT, vC, ST2, 0)
                attention_half(kcT, vC, ST2, 1)
                ln_half(ST2, 0)
                ln_half(ST2, 1)
                # --- feed-forward ---
                ST3 = tpool.tile([P, 8, 5], f32, tag="ST", name="ST")
                for h2 in range(2):
                    for fc in range(NF):
                        pj = ps_pj.tile([P, 512], f32, tag="pj", name="pj")
                        for kc in range(NC):
                            nc.tensor.matmul(pj, w1s[:, kc, fc * P:(fc + 1) * P],
                                             xn[h2][:, xsl(h2, kc)],
                                             start=(kc == 0), stop=(kc == NC - 1))
                        col = l * NF + fc
                        nc.scalar.activation(hT[fc], pj, AF.Relu,
                                             bias=b1_s[:, col:col + 1], scale=1.0)
                    for mc in range(NC):
                        pj = ps_pj.tile([P, 512], f32, tag="pj", name="pj")
                        nc.tensor.matmul(pj, b2_s[0:1, l * C + mc * P:l * C + (mc + 1) * P],
                                         ones_bf[0:1, :], start=True, stop=False)
                        for fc in range(NF):
                            nc.tensor.matmul(pj, w2s[:, fc, mc * P:(mc + 1) * P],
                                             hT[fc],
                                             start=False, stop=(fc == NF - 1))
                        for bb in range(4):
                            b = h2 * 4 + bb
                            dst = xres[h2][:, xsl(h2, mc, bb)]
                            nc.vector.scalar_tensor_tensor(dst,
                                                           pj[:, bb * P:(bb + 1) * P],
                                                           0.0, dst,
                                                           op0=OP.add, op1=OP.add,
                                                           accum_out=ST3[:, b, mc:mc + 1])
                    ln_half(ST3, h2, last=(l == L - 1))

            for k in range(NC):
                for h in range(2):
                    nc.sync.dma_start(
                        out=d_out[k * P:(k + 1) * P, h * 512:(h + 1) * 512],
                        in_=xres[h][:, k * 512:(k + 1) * 512])

    nc.compile()
    return nc


def _prep_shared(inputs):
    """Host-side weight repacking (shared across cores)."""
    def packw(w):  # [L,H,C,DH] -> [L,128,NC,H*DH]  (c = kc*128+p)
        w2 = np.ascontiguousarray(w.transpose(0, 2, 1, 3)).reshape(L, C, HD)
        return np.ascontiguousarray(
            w2.reshape(L, NC, P, HD).transpose(0, 2, 1, 3)).astype(_BF)

    shared = {}
    for nm, key in (("wq", "sa_wq"), ("wk", "sa_wk"), ("wv", "sa_wv"),
                    ("cq", "ca_wq"), ("ck", "ca_wk"), ("cv", "ca_wv")):
        shared[nm] = packw(inputs[key])
    shared["w1"] = np.ascontiguousarray(
        inputs["ff_w1"].reshape(L, NC, P, FF).transpose(0, 2, 1, 3)).astype(_BF)
    shared["w2"] = np.ascontiguousarray(
        inputs["ff_w2"].reshape(L, NF, P, C).transpose(0, 2, 1, 3)).astype(_BF)

    bqk = np.zeros((P, L * 16), np.float32)
    for l in range(L):
        for mi, (bias, s) in enumerate((
                (inputs["sa_bq"][l], 0.125), (inputs["sa_bk"][l], 1.0),
                (inputs["ca_bq"][l], 0.125), (inputs["ca_bk"][l], 1.0))):
            flat = bias.reshape(HD).astype(np.float32) * s
            for mc in range(NC):
                bqk[:, (l * 4 + mi) * 4 + mc] = flat[mc * P:(mc + 1) * P]
    shared["bqk"] = bqk

    bv = np.stack([inputs["sa_bv"].reshape(L, HD),
                   inputs["ca_bv"].reshape(L, HD)], axis=1).astype(np.float32)
    shared["bvb"] = np.ascontiguousarray(
        np.broadcast_to(bv[:, :, None, :], (L, 2, P, HD)))

    b1 = np.zeros((P, L * NF), np.float32)
    for l in range(L):
        for fc in range(NF):
            b1[:, l * NF + fc] = inputs["ff_b1"][l, fc * P:(fc + 1) * P]
    shared["b1"] = b1
    shared["b2"] = inputs["ff_b2"].reshape(1, L * C).astype(_BF)
    return shared


LAST_RESULT = None


def _install_ntff_hook():
    """Register the axon NTFF profile hook that the image's antenv lacks.

    Only used for local benchmarking (KERNEL_TRACE=1); inert otherwise.
    """
    import sys
    import types
    try:
        import antenv
        if getattr(antenv, "axon_hooks", None) is not None:
            return
        from trn_agent_boot.trn_boot import _ntff_profile_via_ctypes
        mod = types.ModuleType("antenv.axon_hooks")
        mod._hook = _ntff_profile_via_ctypes("/opt/axon/libaxon_pjrt.so")

        def get_axon_ntff_profile_hook():
            return mod._hook

        def set_axon_ntff_profile_hook(h):
            mod._hook = h

        mod.get_axon_ntff_profile_hook = get_axon_ntff_profile_hook
        mod.set_axon_ntff_profile_hook = set_axon_ntff_profile_hook
        sys.modules["antenv.axon_hooks"] = mod
        antenv.axon_hooks = mod
    except Exception as e:  # pragma: no cover - profiling is best-effort
        print(f"ntff hook install failed: {e}")


def kernel(**inputs):
    global LAST_RESULT
    import os
    inputs = {k: np.asarray(v) for k, v in inputs.items()}
    if "nc" not in _cache:
        _cache["nc"] = _build()
    nc = _cache["nc"]

    shared = _prep_shared(inputs)
    x = inputs["x"].astype(np.float32)
    enc = inputs["encoder_output"].astype(np.float32)

    in_maps = []
    for core in range(NCORES):
        sl = slice(core * BL, (core + 1) * BL)
        xT = np.ascontiguousarray(x[sl].transpose(2, 0, 1)).reshape(C, TB)
        eT = np.ascontiguousarray(enc[sl].transpose(2, 0, 1)).reshape(C, TB)
        m = dict(shared)
        m["xT"] = xT
        m["xTb"] = xT.astype(_BF)
        m["eT"] = eT.astype(_BF)
        in_maps.append(m)

    trace = bool(int(os.environ.get("KERNEL_TRACE", "0")))
    if trace:
        _install_ntff_hook()
    from concourse.bass_utils import run_bass_kernel_spmd
    res = run_bass_kernel_spmd(nc, in_maps, list(range(NCORES)), trace=trace,
                               trace_cores=[0])
    LAST_RESULT = res

    out = np.empty((B, T, C), np.float32)
    for core in range(NCORES):
        outT = res.results[core]["outT"]  # [C, TB]
        out[core * BL:(core + 1) * BL] = outT.reshape(C, BL, T).transpose(1, 2, 0)
    return out


# revision 30
# speedup vs baseline: 3.1330x; 1.0137x over previous
"""Trainium2 Bass kernel for a 6-layer transformer decoder (self+cross attention).

Strategy: data-parallel over batch across 8 NeuronCores. Each core runs the
full decoder on its 8-batch-item shard, with activations kept transposed
[C, B_local*T] in SBUF so every projection is a natural lhsT.T @ rhs matmul
with a 512-wide moving dim. Matmul inputs are bf16 (fp32 PSUM accumulate);
residual stream and layernorm statistics stay fp32.
"""

import numpy as np
import ml_dtypes

L, H, C, DH, FF = 6, 8, 512, 64, 2048
B, T = 64, 128
EPS = 1e-5
NCORES = 8
BL = B // NCORES          # batch items per core
TB = BL * T               # 1024 activation columns per core
NC = C // 128             # 4 channel chunks
NF = FF // 128            # 16 ff chunks
NPAIR = H // 2            # head pairs
HD = H * DH               # 512
P = 128
NORM = 1.0 / (T * C)      # layernorm 1/N, folded into the stats matmul

_BF = ml_dtypes.bfloat16

_cache = {}


def _build():
    from contextlib import ExitStack

    import concourse.bass as bass  # noqa: F401
    import concourse.tile as tile
    import concourse.mybir as mybir
    from concourse import bacc

    dt = mybir.dt
    AF = mybir.ActivationFunctionType
    OP = mybir.AluOpType
    f32, bf16 = dt.float32, dt.bfloat16

    nc = bacc.Bacc("TRN2", target_bir_lowering=False, debug=False, num_devices=NCORES)

    d_xT = nc.dram_tensor("xT", [C, TB], f32, kind="ExternalInput").ap()
    d_xTb = nc.dram_tensor("xTb", [C, TB], bf16, kind="ExternalInput").ap()
    d_eT = nc.dram_tensor("eT", [C, TB], bf16, kind="ExternalInput").ap()
    d_w = {}
    for name in ("wq", "wk", "wv", "cq", "ck", "cv"):
        d_w[name] = nc.dram_tensor(name, [L, P, NC, HD], bf16, kind="ExternalInput").ap()
    d_w1 = nc.dram_tensor("w1", [L, P, NC, FF], bf16, kind="ExternalInput").ap()
    d_w2 = nc.dram_tensor("w2", [L, P, NF, C], bf16, kind="ExternalInput").ap()
    d_bqk = nc.dram_tensor("bqk", [P, L * 16], f32, kind="ExternalInput").ap()
    d_bvb = nc.dram_tensor("bvb", [L, 2, P, HD], f32, kind="ExternalInput").ap()
    d_b1 = nc.dram_tensor("b1", [P, L * NF], f32, kind="ExternalInput").ap()
    d_b2 = nc.dram_tensor("b2", [P, L * NC], f32, kind="ExternalInput").ap()
    d_out = nc.dram_tensor("outT", [C, TB], f32, kind="ExternalOutput").ap()

    def act_recip(out, in_):
        # ACT-engine reciprocal (~1e-5 rel err measured on hw for this value
        # range); bass's wrapper refuses Reciprocal so emit directly.
        nc.scalar.add_instruction(mybir.InstActivation(
            name=nc.get_next_instruction_name(),
            func=AF.Reciprocal,
            ins=[nc.scalar.lower_ap(in_),
                 mybir.ImmediateValue(dtype=f32, value=0.0),
                 mybir.ImmediateValue(dtype=f32, value=1.0),
                 mybir.ImmediateValue(dtype=f32, value=0.0)],
            outs=[nc.scalar.lower_ap(out)],
        ))

    with tile.TileContext(nc) as tc:
        with ExitStack() as ctx:
            cpool = ctx.enter_context(tc.tile_pool(name="const", bufs=1))
            apool = ctx.enter_context(tc.tile_pool(name="acts", bufs=1))
            wpool = ctx.enter_context(tc.tile_pool(name="wts", bufs=1))
            tpool = ctx.enter_context(tc.tile_pool(name="tmp", bufs=2))
            ps_pj = ctx.enter_context(tc.tile_pool(name="pj", bufs=3, space="PSUM"))
            ps_sc = ctx.enter_context(tc.tile_pool(name="sc", bufs=3, space="PSUM"))
            ps_ao = ctx.enter_context(tc.tile_pool(name="ao", bufs=2, space="PSUM"))

            # ---- constants ----
            ones128b = cpool.tile([P, P], bf16, tag="ones128b")
            nc.vector.memset(ones128b, 1.0)
            onesN = cpool.tile([P, P], f32, tag="onesN")
            nc.vector.memset(onesN, NORM)      # ones/65536 for LN stats matmul
            eps_t = cpool.tile([P, 1], f32, tag="eps")
            nc.vector.memset(eps_t, EPS)
            zero_t = cpool.tile([P, 1], f32, tag="zero")
            nc.vector.memset(zero_t, 0.0)
            bqk_s = cpool.tile([P, L * 16], f32, tag="bqk")
            nc.sync.dma_start(out=bqk_s, in_=d_bqk)
            b1_s = cpool.tile([P, L * NF], f32, tag="b1")
            nc.sync.dma_start(out=b1_s, in_=d_b1)
            b2_s = cpool.tile([P, L * NC], f32, tag="b2")
            nc.sync.dma_start(out=b2_s, in_=d_b2)

            # ---- persistent activations (kc-major merged tiles per half) ----
            xres = [apool.tile([P, NC * 512], f32, tag=f"xres{h}", name=f"xres{h}")
                    for h in range(2)]
            xn = [apool.tile([P, NC * 512], bf16, tag=f"xn{h}", name=f"xn{h}")
                  for h in range(2)]
            eTs = [apool.tile([P, NC * 512], bf16, tag=f"eT{h}", name=f"eT{h}")
                   for h in range(2)]
            for k in range(NC):
                for h in range(2):
                    rs = slice(k * P, (k + 1) * P)
                    cs_ = slice(h * 512, (h + 1) * 512)
                    ts_ = slice(k * 512, (k + 1) * 512)
                    nc.sync.dma_start(out=xres[h][:, ts_], in_=d_xT[rs, cs_])
                    nc.sync.dma_start(out=xn[h][:, ts_], in_=d_xTb[rs, cs_])
                    nc.sync.dma_start(out=eTs[h][:, ts_], in_=d_eT[rs, cs_])

            def xsl(h2, kc, bb=None):
                if bb is None:
                    return slice(kc * 512, (kc + 1) * 512)
                return slice(kc * 512 + bb * P, kc * 512 + (bb + 1) * P)

            qT = [apool.tile([P, TB], bf16, tag=f"qT{k}", name=f"qT{k}") for k in range(NC)]
            kT = [apool.tile([P, TB], bf16, tag=f"kT{k}", name=f"kT{k}") for k in range(NC)]
            kcT = [apool.tile([P, TB], bf16, tag=f"kcT{k}", name=f"kcT{k}") for k in range(NC)]
            vS = [apool.tile([P, HD], bf16, tag=f"v{b}", name=f"v{b}") for b in range(BL)]
            vC = [apool.tile([P, HD], bf16, tag=f"vc{b}", name=f"vc{b}") for b in range(BL)]
            hT = [apool.tile([P, 512], bf16, tag=f"hT{f}", name=f"hT{f}") for f in range(NF)]

            def proj_qk(dst, wt, src, col_of, scale, halves=(0, 1)):
                for h2 in halves:
                    for mc in range(NC):
                        pj = ps_pj.tile([P, 512], f32, tag="pj", name="pj")
                        for kc in range(NC):
                            nc.tensor.matmul(pj, wt[:, kc, mc * P:(mc + 1) * P],
                                             src[h2][:, xsl(h2, kc)],
                                             start=(kc == 0), stop=(kc == NC - 1))
                        col = col_of(mc)
                        nc.scalar.activation(dst[mc][:, h2 * 512:(h2 + 1) * 512], pj,
                                             AF.Identity,
                                             bias=bqk_s[:, col:col + 1], scale=scale)

            def proj_v(dst, wt, src, bias_tile, bs):
                for b in bs:
                    h2, bb = divmod(b, 4)
                    pj = ps_pj.tile([P, 512], f32, tag="pj", name="pj")
                    for kc in range(NC):
                        nc.tensor.matmul(pj, src[h2][:, xsl(h2, kc, bb)],
                                         wt[:, kc, :],
                                         start=(kc == 0), stop=(kc == NC - 1))
                    nc.vector.tensor_tensor(dst[b], pj, bias_tile, op=OP.add)

            def attention_half(kTl, vl, ST, h2, filler=None):
                # Per half (4 batch items), in 2-batch groups, with ACT ops
                # batched by function to avoid activation-table reloads:
                # all scores -> all exp -> all colsum-bcast matmuls -> all
                # reciprocals -> all normalizes -> all attn-out + evict.
                expTs = {}
                pending = []
                for bb in range(4):
                    b = h2 * 4 + bb
                    expT = tpool.tile([P, TB], bf16, tag="expT", name="expT",
                                      bufs=4)
                    sce = ps_sc.tile([P, 512], f32, tag="sc", name="sc")
                    sco = ps_sc.tile([P, 512], f32, tag="sc", name="sc")
                    for p in range(NPAIR):
                        nc.tensor.matmul(sce[:, p * P:(p + 1) * P],
                                         kTl[p][0:64, b * P:(b + 1) * P],
                                         qT[p][0:64, b * P:(b + 1) * P],
                                         start=True, stop=True,
                                         tile_position=(0, 0))
                        nc.tensor.matmul(sco[:, p * P:(p + 1) * P],
                                         kTl[p][64:128, b * P:(b + 1) * P],
                                         qT[p][64:128, b * P:(b + 1) * P],
                                         start=True, stop=True,
                                         tile_position=(64, 0))
                    pending.append((bb, expT, sce, sco))
                    expTs[bb] = expT
                    if len(pending) == 2 or bb == 3:
                        for _bb, _e, _sce, _sco in pending:
                            nc.scalar.activation(_e[:, 0:512], _sce, AF.Exp,
                                                 bias=zero_t, scale=1.0)
                            nc.scalar.activation(_e[:, 512:1024], _sco, AF.Exp,
                                                 bias=zero_t, scale=1.0)
                        pending = []
                # colsum broadcast matmuls, then all reciprocals, then mults
                ddrs = {}
                for bb in range(4):
                    for j in range(2):
                        sl = slice(j * 512, (j + 1) * 512)
                        dsum = ps_sc.tile([P, 512], f32, tag="sc", name="sc")
                        nc.tensor.matmul(dsum, ones128b, expTs[bb][:, sl],
                                         start=True, stop=True)
                        ddr = tpool.tile([P, 512], bf16, tag="ddr", name="ddr",
                                         bufs=6)
                        act_recip(ddr, dsum)
                        ddrs[(bb, j)] = ddr
                if filler is not None:
                    filler()
                for bb in range(4):
                    for j in range(2):
                        sl = slice(j * 512, (j + 1) * 512)
                        nc.vector.tensor_tensor(expTs[bb][:, sl],
                                                expTs[bb][:, sl],
                                                ddrs[(bb, j)], op=OP.mult)
                for bb in range(4):
                    b = h2 * 4 + bb
                    for p in range(NPAIR):
                        ao = ps_ao.tile([P, P], f32, tag="ao", name="ao")
                        for j in range(2):
                            h = 2 * p + j
                            pos = (h % 2) * 512 + (h // 2) * P
                            nc.tensor.matmul(ao[j * 64:(j + 1) * 64, :],
                                             vl[b][:, h * 64:(h + 1) * 64],
                                             expTs[bb][:, pos:pos + P],
                                             start=True, stop=True,
                                             tile_position=(0, j * 64))
                        dst = xres[h2][:, xsl(h2, p, bb)]
                        nc.vector.scalar_tensor_tensor(dst, ao, 0.0, dst,
                                                       op0=OP.add, op1=OP.add,
                                                       accum_out=ST[:, b, p:p + 1])

            def ln_half(ST, h2, last=False):
                # stats per batch item over (T, C); per-chunk sums were
                # accumulated into ST[:, b, 0:4] by the residual-evict ops;
                # a single fused square+reduce per item fills ST[:, b, 4].
                x3 = xres[h2].rearrange("p (k n) -> p k n", k=NC)
                for bb in range(4):
                    b = h2 * 4 + bb
                    sq = tpool.tile([P, 512], bf16, tag="sq", name="sq")
                    src = x3[:, :, bb * P:(bb + 1) * P]
                    nc.vector.scalar_tensor_tensor(
                        sq.rearrange("p (k n) -> p k n", k=NC), src, 1.0, src,
                        op0=OP.mult, op1=OP.mult,
                        accum_out=ST[:, b, 4:5])
                tot = ps_pj.tile([P, 20], f32, tag="pj", name="pj")
                nc.tensor.matmul(tot, onesN,
                                 ST[:, h2 * 4:(h2 + 1) * 4, :].rearrange("p a b -> p (a b)"),
                                 start=True, stop=True)
                tot3 = tot.rearrange("p (a b) -> p a b", b=5)
                mm_ = tpool.tile([P, 4], f32, tag="mm_", name="mm_")
                nc.vector.reduce_sum(mm_, tot3[:, :, 0:4], axis=mybir.AxisListType.X)
                var = tpool.tile([P, 4], f32, tag="var", name="var")
                nc.vector.tensor_tensor(var, mm_, mm_, op=OP.mult)
                nc.vector.tensor_tensor(var, tot3[:, :, 4], var, op=OP.subtract)
                sd = tpool.tile([P, 4], f32, tag="sd", name="sd")
                nc.scalar.activation(sd, var, AF.Sqrt, bias=eps_t, scale=1.0)
                rr = tpool.tile([P, 4], f32, tag="rr", name="rr")
                nc.vector.reciprocal(rr, sd)
                for bb in range(4):
                    src = x3[:, :, bb * P:(bb + 1) * P]
                    nc.vector.tensor_scalar(src, src, mm_[:, bb:bb + 1],
                                            rr[:, bb:bb + 1],
                                            op0=OP.subtract, op1=OP.mult)
                if not last:
                    for kc in range(NC):
                        sl = slice(kc * 512, (kc + 1) * 512)
                        nc.vector.tensor_copy(out=xn[h2][:, sl],
                                              in_=xres[h2][:, sl])

            for l in range(L):
                wts = {}
                for name in ("wq", "wk", "wv", "cq", "ck", "cv"):
                    w = wpool.tile([P, NC, HD], bf16, tag=name, name=name)
                    nc.sync.dma_start(out=w, in_=d_w[name][l])
                    wts[name] = w
                w1s = wpool.tile([P, NC, FF], bf16, tag="w1", name="w1")
                nc.sync.dma_start(out=w1s, in_=d_w1[l])
                w2s = wpool.tile([P, NF, C], bf16, tag="w2", name="w2")
                nc.sync.dma_start(out=w2s, in_=d_w2[l])
                bvs = wpool.tile([P, HD], f32, tag="bvs", name="bvs")
                nc.sync.dma_start(out=bvs, in_=d_bvb[l, 0])
                bvc = wpool.tile([P, HD], f32, tag="bvc", name="bvc")
                nc.sync.dma_start(out=bvc, in_=d_bvb[l, 1])

                # --- self attention (cross K/V emitted as PE filler) ---
                ST1 = tpool.tile([P, 8, 5], f32, tag="ST", name="ST")
                proj_qk(qT, wts["wq"], xn, lambda mc: (l * 4 + 0) * 4 + mc, 0.125)
                proj_qk(kT, wts["wk"], xn, lambda mc: (l * 4 + 1) * 4 + mc, 1.0)
                proj_v(vS, wts["wv"], xn, bvs, range(BL))

                def filler0():
                    proj_qk(kcT, wts["ck"], eTs,
                            lambda mc: (l * 4 + 3) * 4 + mc, 1.0, halves=(0,))
                    proj_v(vC, wts["cv"], eTs, bvc, range(0, 4))

                def filler1():
                    proj_qk(kcT, wts["ck"], eTs,
                            lambda mc: (l * 4 + 3) * 4 + mc, 1.0, halves=(1,))
                    proj_v(vC, wts["cv"], eTs, bvc, range(4, 8))

                attention_half(kT, vS, ST1, 0, filler=filler0)
                attention_half(kT, vS, ST1, 1, filler=filler1)
                ln_half(ST1, 0)
                ln_half(ST1, 1)
                # --- cross attention ---
                ST2 = tpool.tile([P, 8, 5], f32, tag="ST", name="ST")
                proj_qk(qT, wts["cq"], xn, lambda mc: (l * 4 + 2) * 4 + mc, 0.125)
                attention_half(kcT, vC, ST2, 0)
                attention_half(kcT, vC, ST2, 1)
                ln_half(ST2, 0)
                ln_half(ST2, 1)
                # --- feed-forward ---
                ST3 = tpool.tile([P, 8, 5], f32, tag="ST", name="ST")
                for h2 in range(2):
                    for fc in range(NF):
                        pj = ps_pj.tile([P, 512], f32, tag="pj", name="pj")
                        for kc in range(NC):
                            nc.tensor.matmul(pj, w1s[:, kc, fc * P:(fc + 1) * P],
                                             xn[h2][:, xsl(h2, kc)],
                                             start=(kc == 0), stop=(kc == NC - 1))
                        col = l * NF + fc
                        nc.scalar.activation(hT[fc], pj, AF.Relu,
                                             bias=b1_s[:, col:col + 1], scale=1.0)
                    for mc in range(NC):
                        pj = ps_pj.tile([P, 512], f32, tag="pj", name="pj")
                        for fc in range(NF):
                            nc.tensor.matmul(pj, w2s[:, fc, mc * P:(mc + 1) * P],
                                             hT[fc],
                                             start=(fc == 0), stop=(fc == NF - 1))
                        b2col = b2_s[:, l * NC + mc:l * NC + mc + 1]
                        for bb in range(4):
                            b = h2 * 4 + bb
                            dst = xres[h2][:, xsl(h2, mc, bb)]
                            nc.vector.scalar_tensor_tensor(dst,
                                                           pj[:, bb * P:(bb + 1) * P],
                                                           b2col, dst,
                                                           op0=OP.add, op1=OP.add,
                                                           accum_out=ST3[:, b, mc:mc + 1])
                    ln_half(ST3, h2, last=(l == L - 1))

            for k in range(NC):
                for h in range(2):
                    nc.sync.dma_start(
                        out=d_out[k * P:(k + 1) * P, h * 512:(h + 1) * 512],
                        in_=xres[h][:, k * 512:(k + 1) * 512])

    nc.compile()
    return nc


def _prep_shared(inputs):
    """Host-side weight repacking (shared across cores)."""
    def packw(w):  # [L,H,C,DH] -> [L,128,NC,H*DH]  (c = kc*128+p)
        w2 = np.ascontiguousarray(w.transpose(0, 2, 1, 3)).reshape(L, C, HD)
        return np.ascontiguousarray(
            w2.reshape(L, NC, P, HD).transpose(0, 2, 1, 3)).astype(_BF)

    shared = {}
    for nm, key in (("wq", "sa_wq"), ("wk", "sa_wk"), ("wv", "sa_wv"),
                    ("cq", "ca_wq"), ("ck", "ca_wk"), ("cv", "ca_wv")):
        shared[nm] = packw(inputs[key])
    shared["w1"] = np.ascontiguousarray(
        inputs["ff_w1"].reshape(L, NC, P, FF).transpose(0, 2, 1, 3)).astype(_BF)
    shared["w2"] = np.ascontiguousarray(
        inputs["ff_w2"].reshape(L, NF, P, C).transpose(0, 2, 1, 3)).astype(_BF)

    bqk = np.zeros((P, L * 16), np.float32)
    for l in range(L):
        for mi, (bias, s) in enumerate((
                (inputs["sa_bq"][l], 0.125), (inputs["sa_bk"][l], 1.0),
                (inputs["ca_bq"][l], 0.125), (inputs["ca_bk"][l], 1.0))):
            flat = bias.reshape(HD).astype(np.float32) * s
            for mc in range(NC):
                bqk[:, (l * 4 + mi) * 4 + mc] = flat[mc * P:(mc + 1) * P]
    shared["bqk"] = bqk

    bv = np.stack([inputs["sa_bv"].reshape(L, HD),
                   inputs["ca_bv"].reshape(L, HD)], axis=1).astype(np.float32)
    shared["bvb"] = np.ascontiguousarray(
        np.broadcast_to(bv[:, :, None, :], (L, 2, P, HD)))

    b1 = np.zeros((P, L * NF), np.float32)
    for l in range(L):
        for fc in range(NF):
            b1[:, l * NF + fc] = inputs["ff_b1"][l, fc * P:(fc + 1) * P]
    shared["b1"] = b1
    b2 = np.zeros((P, L * NC), np.float32)
    for l in range(L):
        for mc in range(NC):
            b2[:, l * NC + mc] = inputs["ff_b2"][l, mc * P:(mc + 1) * P]
    shared["b2"] = b2
    return shared


LAST_RESULT = None


def _install_ntff_hook():
    """Register the axon NTFF profile hook that the image's antenv lacks.

    Only used for local benchmarking (KERNEL_TRACE=1); inert otherwise.
    """
    import sys
    import types
    try:
        import antenv
        if getattr(antenv, "axon_hooks", None) is not None:
            return
        from trn_agent_boot.trn_boot import _ntff_profile_via_ctypes
        mod = types.ModuleType("antenv.axon_hooks")
        mod._hook = _ntff_profile_via_ctypes("/opt/axon/libaxon_pjrt.so")

        def get_axon_ntff_profile_hook():
            return mod._hook

        def set_axon_ntff_profile_hook(h):
            mod._hook = h

        mod.get_axon_ntff_profile_hook = get_axon_ntff_profile_hook
        mod.set_axon_ntff_profile_hook = set_axon_ntff_profile_hook
        sys.modules["antenv.axon_hooks"] = mod
        antenv.axon_hooks = mod
    except Exception as e:  # pragma: no cover - profiling is best-effort
        print(f"ntff hook install failed: {e}")


def kernel(**inputs):
    global LAST_RESULT
    import os
    inputs = {k: np.asarray(v) for k, v in inputs.items()}
    if "nc" not in _cache:
        _cache["nc"] = _build()
    nc = _cache["nc"]

    shared = _prep_shared(inputs)
    x = inputs["x"].astype(np.float32)
    enc = inputs["encoder_output"].astype(np.float32)

    in_maps = []
    for core in range(NCORES):
        sl = slice(core * BL, (core + 1) * BL)
        xT = np.ascontiguousarray(x[sl].transpose(2, 0, 1)).reshape(C, TB)
        eT = np.ascontiguousarray(enc[sl].transpose(2, 0, 1)).reshape(C, TB)
        m = dict(shared)
        m["xT"] = xT
        m["xTb"] = xT.astype(_BF)
        m["eT"] = eT.astype(_BF)
        in_maps.append(m)

    trace = bool(int(os.environ.get("KERNEL_TRACE", "0")))
    if trace:
        _install_ntff_hook()
    from concourse.bass_utils import run_bass_kernel_spmd
    res = run_bass_kernel_spmd(nc, in_maps, list(range(NCORES)), trace=trace,
                               trace_cores=[0])
    LAST_RESULT = res

    out = np.empty((B, T, C), np.float32)
    for core in range(NCORES):
        outT = res.results[core]["outT"]  # [C, TB]
        out[core * BL:(core + 1) * BL] = outT.reshape(C, BL, T).transpose(1, 2, 0)
    return out


# revision 31
# speedup vs baseline: 3.3406x; 1.0663x over previous
"""Trainium2 Bass kernel for a 6-layer transformer decoder (self+cross attention).

Strategy: data-parallel over batch across 8 NeuronCores. Each core runs the
full decoder on its 8-batch-item shard, with activations kept transposed
[C, B_local*T] in SBUF so every projection is a natural lhsT.T @ rhs matmul
with a 512-wide moving dim. Matmul inputs are bf16 (fp32 PSUM accumulate);
residual stream and layernorm statistics stay fp32.
"""

import numpy as np
import ml_dtypes

L, H, C, DH, FF = 6, 8, 512, 64, 2048
B, T = 64, 128
EPS = 1e-5
NCORES = 8
BL = B // NCORES          # batch items per core
TB = BL * T               # 1024 activation columns per core
NC = C // 128             # 4 channel chunks
NF = FF // 128            # 16 ff chunks
NPAIR = H // 2            # head pairs
HD = H * DH               # 512
P = 128
NORM = 1.0 / (T * C)      # layernorm 1/N, folded into the stats matmul

_BF = ml_dtypes.bfloat16

_cache = {}


def _build():
    from contextlib import ExitStack

    import concourse.bass as bass  # noqa: F401
    import concourse.tile as tile
    import concourse.mybir as mybir
    from concourse import bacc

    dt = mybir.dt
    AF = mybir.ActivationFunctionType
    OP = mybir.AluOpType
    f32, bf16 = dt.float32, dt.bfloat16

    nc = bacc.Bacc("TRN2", target_bir_lowering=False, debug=False, num_devices=NCORES)

    d_xT = nc.dram_tensor("xT", [C, TB], f32, kind="ExternalInput").ap()
    d_xTb = nc.dram_tensor("xTb", [C, TB], bf16, kind="ExternalInput").ap()
    d_eT = nc.dram_tensor("eT", [C, TB], bf16, kind="ExternalInput").ap()
    d_w = {}
    for name in ("wq", "wk", "wv", "cq", "ck", "cv"):
        d_w[name] = nc.dram_tensor(name, [L, P, NC, HD], bf16, kind="ExternalInput").ap()
    d_w1 = nc.dram_tensor("w1", [L, P, NC, FF], bf16, kind="ExternalInput").ap()
    d_w2 = nc.dram_tensor("w2", [L, P, NF, C], bf16, kind="ExternalInput").ap()
    d_bqk = nc.dram_tensor("bqk", [P, L * 16], f32, kind="ExternalInput").ap()
    d_bvb = nc.dram_tensor("bvb", [L, 2, P, HD], f32, kind="ExternalInput").ap()
    d_b1 = nc.dram_tensor("b1", [P, L * NF], f32, kind="ExternalInput").ap()
    d_b2 = nc.dram_tensor("b2", [P, L * NC], f32, kind="ExternalInput").ap()
    d_out = nc.dram_tensor("outT", [C, TB], f32, kind="ExternalOutput").ap()

    def act_recip(out, in_):
        # ACT-engine reciprocal (~1e-5 rel err measured on hw for this value
        # range); bass's wrapper refuses Reciprocal so emit directly.
        nc.scalar.add_instruction(mybir.InstActivation(
            name=nc.get_next_instruction_name(),
            func=AF.Reciprocal,
            ins=[nc.scalar.lower_ap(in_),
                 mybir.ImmediateValue(dtype=f32, value=0.0),
                 mybir.ImmediateValue(dtype=f32, value=1.0),
                 mybir.ImmediateValue(dtype=f32, value=0.0)],
            outs=[nc.scalar.lower_ap(out)],
        ))

    with tile.TileContext(nc) as tc:
        with ExitStack() as ctx:
            cpool = ctx.enter_context(tc.tile_pool(name="const", bufs=1))
            apool = ctx.enter_context(tc.tile_pool(name="acts", bufs=1))
            wpool = ctx.enter_context(tc.tile_pool(name="wts", bufs=1))
            tpool = ctx.enter_context(tc.tile_pool(name="tmp", bufs=2))
            ps_pj = ctx.enter_context(tc.tile_pool(name="pj", bufs=3, space="PSUM"))
            ps_sc = ctx.enter_context(tc.tile_pool(name="sc", bufs=3, space="PSUM"))
            ps_ao = ctx.enter_context(tc.tile_pool(name="ao", bufs=2, space="PSUM"))

            # ---- constants ----
            ones128b = cpool.tile([P, P], bf16, tag="ones128b")
            nc.vector.memset(ones128b, 1.0)
            onesN = cpool.tile([P, P], f32, tag="onesN")
            nc.vector.memset(onesN, NORM)      # ones/65536 for LN stats matmul
            eps_t = cpool.tile([P, 1], f32, tag="eps")
            nc.vector.memset(eps_t, EPS)
            zero_t = cpool.tile([P, 1], f32, tag="zero")
            nc.vector.memset(zero_t, 0.0)
            bqk_s = cpool.tile([P, L * 16], f32, tag="bqk")
            nc.sync.dma_start(out=bqk_s, in_=d_bqk)
            b1_s = cpool.tile([P, L * NF], f32, tag="b1")
            nc.sync.dma_start(out=b1_s, in_=d_b1)
            b2_s = cpool.tile([P, L * NC], f32, tag="b2")
            nc.sync.dma_start(out=b2_s, in_=d_b2)

            # ---- persistent activations (kc-major merged tiles per half) ----
            xres = [apool.tile([P, NC * 512], f32, tag=f"xres{h}", name=f"xres{h}")
                    for h in range(2)]
            xn = [apool.tile([P, NC * 512], bf16, tag=f"xn{h}", name=f"xn{h}")
                  for h in range(2)]
            eTs = [apool.tile([P, NC * 512], bf16, tag=f"eT{h}", name=f"eT{h}")
                   for h in range(2)]
            for k in range(NC):
                for h in range(2):
                    rs = slice(k * P, (k + 1) * P)
                    cs_ = slice(h * 512, (h + 1) * 512)
                    ts_ = slice(k * 512, (k + 1) * 512)
                    nc.sync.dma_start(out=xres[h][:, ts_], in_=d_xT[rs, cs_])
                    nc.sync.dma_start(out=xn[h][:, ts_], in_=d_xTb[rs, cs_])
                    nc.sync.dma_start(out=eTs[h][:, ts_], in_=d_eT[rs, cs_])

            def xsl(h2, kc, bb=None):
                if bb is None:
                    return slice(kc * 512, (kc + 1) * 512)
                return slice(kc * 512 + bb * P, kc * 512 + (bb + 1) * P)

            qT = [apool.tile([P, TB], bf16, tag=f"qT{k}", name=f"qT{k}") for k in range(NC)]
            kT = [apool.tile([P, TB], bf16, tag=f"kT{k}", name=f"kT{k}") for k in range(NC)]
            kcT = [apool.tile([P, TB], bf16, tag=f"kcT{k}", name=f"kcT{k}") for k in range(NC)]
            vS = [apool.tile([P, HD], bf16, tag=f"v{b}", name=f"v{b}") for b in range(BL)]
            vC = [apool.tile([P, HD], bf16, tag=f"vc{b}", name=f"vc{b}") for b in range(BL)]
            hT = [apool.tile([P, 512], bf16, tag=f"hT{f}", name=f"hT{f}") for f in range(NF)]

            def proj_qk(dst, wt, src, col_of, scale, halves=(0, 1)):
                for h2 in halves:
                    for mc in range(NC):
                        pj = ps_pj.tile([P, 512], f32, tag="pj", name="pj")
                        for kc in range(NC):
                            nc.tensor.matmul(pj, wt[:, kc, mc * P:(mc + 1) * P],
                                             src[h2][:, xsl(h2, kc)],
                                             start=(kc == 0), stop=(kc == NC - 1))
                        col = col_of(mc)
                        nc.scalar.activation(dst[mc][:, h2 * 512:(h2 + 1) * 512], pj,
                                             AF.Identity,
                                             bias=bqk_s[:, col:col + 1], scale=scale)

            def proj_v(dst, wt, src, bias_tile, bs):
                for b in bs:
                    h2, bb = divmod(b, 4)
                    pj = ps_pj.tile([P, 512], f32, tag="pj", name="pj")
                    for kc in range(NC):
                        nc.tensor.matmul(pj, src[h2][:, xsl(h2, kc, bb)],
                                         wt[:, kc, :],
                                         start=(kc == 0), stop=(kc == NC - 1))
                    nc.vector.tensor_tensor(dst[b], pj, bias_tile, op=OP.add)

            def attention_half(kTl, vl, ST, h2, filler=None):
                # Per half (4 batch items), in 2-batch groups, with ACT ops
                # batched by function to avoid activation-table reloads:
                # all scores -> all exp -> all colsum-bcast matmuls -> all
                # reciprocals -> all normalizes -> all attn-out + evict.
                expTs = {}
                pending = []
                for bb in range(4):
                    b = h2 * 4 + bb
                    expT = tpool.tile([P, TB], bf16, tag="expT", name="expT",
                                      bufs=4)
                    sce = ps_sc.tile([P, 512], f32, tag="sc", name="sc")
                    sco = ps_sc.tile([P, 512], f32, tag="sc", name="sc")
                    for p in range(NPAIR):
                        nc.tensor.matmul(sce[:, p * P:(p + 1) * P],
                                         kTl[p][0:64, b * P:(b + 1) * P],
                                         qT[p][0:64, b * P:(b + 1) * P],
                                         start=True, stop=True,
                                         tile_position=(0, 0))
                        nc.tensor.matmul(sco[:, p * P:(p + 1) * P],
                                         kTl[p][64:128, b * P:(b + 1) * P],
                                         qT[p][64:128, b * P:(b + 1) * P],
                                         start=True, stop=True,
                                         tile_position=(64, 0))
                    pending.append((bb, expT, sce, sco))
                    expTs[bb] = expT
                    if len(pending) == 2 or bb == 3:
                        for _bb, _e, _sce, _sco in pending:
                            nc.scalar.activation(_e[:, 0:512], _sce, AF.Exp,
                                                 bias=zero_t, scale=1.0)
                            nc.scalar.activation(_e[:, 512:1024], _sco, AF.Exp,
                                                 bias=zero_t, scale=1.0)
                        pending = []
                # colsum broadcast matmuls, then all reciprocals, then mults
                ddrs = {}
                for bb in range(4):
                    for j in range(2):
                        sl = slice(j * 512, (j + 1) * 512)
                        dsum = ps_sc.tile([P, 512], f32, tag="sc", name="sc")
                        nc.tensor.matmul(dsum, ones128b, expTs[bb][:, sl],
                                         start=True, stop=True)
                        ddr = tpool.tile([P, 512], bf16, tag="ddr", name="ddr",
                                         bufs=6)
                        act_recip(ddr, dsum)
                        ddrs[(bb, j)] = ddr
                if filler is not None:
                    filler()
                for bb in range(4):
                    for j in range(2):
                        sl = slice(j * 512, (j + 1) * 512)
                        nc.vector.tensor_tensor(expTs[bb][:, sl],
                                                expTs[bb][:, sl],
                                                ddrs[(bb, j)], op=OP.mult)
                x3 = xres[h2].rearrange("p (k n) -> p k n", k=NC)
                for bb in range(4):
                    b = h2 * 4 + bb
                    ao = ps_ao.tile([P, 512], f32, tag="ao", name="ao")
                    for p in range(NPAIR):
                        for j in range(2):
                            h = 2 * p + j
                            pos = (h % 2) * 512 + (h // 2) * P
                            nc.tensor.matmul(ao[j * 64:(j + 1) * 64, p * P:(p + 1) * P],
                                             vl[b][:, h * 64:(h + 1) * 64],
                                             expTs[bb][:, pos:pos + P],
                                             start=True, stop=True,
                                             tile_position=(0, j * 64))
                    dst = x3[:, :, bb * P:(bb + 1) * P]
                    nc.vector.scalar_tensor_tensor(dst,
                                                   ao.rearrange("p (k n) -> p k n", k=NC),
                                                   0.0, dst,
                                                   op0=OP.add, op1=OP.add,
                                                   accum_out=ST[:, b, 0:1])

            def ln_half(ST, h2, last=False, nsum=1):
                # stats per batch item over (T, C); sums were accumulated
                # into ST[:, b, 0:nsum] by the residual-evict ops; a single
                # fused square+reduce per item fills ST[:, b, 4].
                x3 = xres[h2].rearrange("p (k n) -> p k n", k=NC)
                for bb in range(4):
                    b = h2 * 4 + bb
                    sq = tpool.tile([P, 512], bf16, tag="sq", name="sq")
                    src = x3[:, :, bb * P:(bb + 1) * P]
                    nc.vector.scalar_tensor_tensor(
                        sq.rearrange("p (k n) -> p k n", k=NC), src, 1.0, src,
                        op0=OP.mult, op1=OP.mult,
                        accum_out=ST[:, b, 4:5])
                tot = ps_pj.tile([P, 20], f32, tag="pj", name="pj")
                nc.tensor.matmul(tot, onesN,
                                 ST[:, h2 * 4:(h2 + 1) * 4, :].rearrange("p a b -> p (a b)"),
                                 start=True, stop=True)
                tot3 = tot.rearrange("p (a b) -> p a b", b=5)
                mm_ = tpool.tile([P, 4], f32, tag="mm_", name="mm_")
                if nsum == 1:
                    nc.vector.tensor_copy(out=mm_, in_=tot3[:, :, 0])
                else:
                    nc.vector.reduce_sum(mm_, tot3[:, :, 0:nsum],
                                         axis=mybir.AxisListType.X)
                var = tpool.tile([P, 4], f32, tag="var", name="var")
                nc.vector.tensor_tensor(var, mm_, mm_, op=OP.mult)
                nc.vector.tensor_tensor(var, tot3[:, :, 4], var, op=OP.subtract)
                sd = tpool.tile([P, 4], f32, tag="sd", name="sd")
                nc.scalar.activation(sd, var, AF.Sqrt, bias=eps_t, scale=1.0)
                rr = tpool.tile([P, 4], f32, tag="rr", name="rr")
                nc.vector.reciprocal(rr, sd)
                for bb in range(4):
                    src = x3[:, :, bb * P:(bb + 1) * P]
                    nc.vector.tensor_scalar(src, src, mm_[:, bb:bb + 1],
                                            rr[:, bb:bb + 1],
                                            op0=OP.subtract, op1=OP.mult)
                if not last:
                    for kc in range(NC):
                        sl = slice(kc * 512, (kc + 1) * 512)
                        nc.vector.tensor_copy(out=xn[h2][:, sl],
                                              in_=xres[h2][:, sl])

            for l in range(L):
                wts = {}
                for name in ("wq", "wk", "wv", "cq", "ck", "cv"):
                    w = wpool.tile([P, NC, HD], bf16, tag=name, name=name)
                    nc.sync.dma_start(out=w, in_=d_w[name][l])
                    wts[name] = w
                w1s = wpool.tile([P, NC, FF], bf16, tag="w1", name="w1")
                nc.sync.dma_start(out=w1s, in_=d_w1[l])
                w2s = wpool.tile([P, NF, C], bf16, tag="w2", name="w2")
                nc.sync.dma_start(out=w2s, in_=d_w2[l])
                bvs = wpool.tile([P, HD], f32, tag="bvs", name="bvs")
                nc.sync.dma_start(out=bvs, in_=d_bvb[l, 0])
                bvc = wpool.tile([P, HD], f32, tag="bvc", name="bvc")
                nc.sync.dma_start(out=bvc, in_=d_bvb[l, 1])

                # --- self attention (cross K/V emitted as PE filler) ---
                ST1 = tpool.tile([P, 8, 5], f32, tag="ST", name="ST")
                proj_qk(qT, wts["wq"], xn, lambda mc: (l * 4 + 0) * 4 + mc, 0.125)
                proj_qk(kT, wts["wk"], xn, lambda mc: (l * 4 + 1) * 4 + mc, 1.0)
                proj_v(vS, wts["wv"], xn, bvs, range(BL))

                def filler0():
                    proj_qk(kcT, wts["ck"], eTs,
                            lambda mc: (l * 4 + 3) * 4 + mc, 1.0, halves=(0,))
                    proj_v(vC, wts["cv"], eTs, bvc, range(0, 4))

                def filler1():
                    proj_qk(kcT, wts["ck"], eTs,
                            lambda mc: (l * 4 + 3) * 4 + mc, 1.0, halves=(1,))
                    proj_v(vC, wts["cv"], eTs, bvc, range(4, 8))

                attention_half(kT, vS, ST1, 0, filler=filler0)
                attention_half(kT, vS, ST1, 1, filler=filler1)
                ln_half(ST1, 0)
                ln_half(ST1, 1)
                # --- cross attention ---
                ST2 = tpool.tile([P, 8, 5], f32, tag="ST", name="ST")
                proj_qk(qT, wts["cq"], xn, lambda mc: (l * 4 + 2) * 4 + mc, 0.125)
                attention_half(kcT, vC, ST2, 0)
                attention_half(kcT, vC, ST2, 1)
                ln_half(ST2, 0)
                ln_half(ST2, 1)
                # --- feed-forward ---
                ST3 = tpool.tile([P, 8, 5], f32, tag="ST", name="ST")
                for h2 in range(2):
                    for fc in range(NF):
                        pj = ps_pj.tile([P, 512], f32, tag="pj", name="pj")
                        for kc in range(NC):
                            nc.tensor.matmul(pj, w1s[:, kc, fc * P:(fc + 1) * P],
                                             xn[h2][:, xsl(h2, kc)],
                                             start=(kc == 0), stop=(kc == NC - 1))
                        col = l * NF + fc
                        nc.scalar.activation(hT[fc], pj, AF.Relu,
                                             bias=b1_s[:, col:col + 1], scale=1.0)
                    for mc in range(NC):
                        pj = ps_pj.tile([P, 512], f32, tag="pj", name="pj")
                        for fc in range(NF):
                            nc.tensor.matmul(pj, w2s[:, fc, mc * P:(mc + 1) * P],
                                             hT[fc],
                                             start=(fc == 0), stop=(fc == NF - 1))
                        b2col = b2_s[:, l * NC + mc:l * NC + mc + 1]
                        for bb in range(4):
                            b = h2 * 4 + bb
                            dst = xres[h2][:, xsl(h2, mc, bb)]
                            nc.vector.scalar_tensor_tensor(dst,
                                                           pj[:, bb * P:(bb + 1) * P],
                                                           b2col, dst,
                                                           op0=OP.add, op1=OP.add,
                                                           accum_out=ST3[:, b, mc:mc + 1])
                    ln_half(ST3, h2, last=(l == L - 1), nsum=4)

            for k in range(NC):
                for h in range(2):
                    nc.sync.dma_start(
                        out=d_out[k * P:(k + 1) * P, h * 512:(h + 1) * 512],
                        in_=xres[h][:, k * 512:(k + 1) * 512])

    nc.compile()
    return nc


def _prep_shared(inputs):
    """Host-side weight repacking (shared across cores)."""
    def packw(w):  # [L,H,C,DH] -> [L,128,NC,H*DH]  (c = kc*128+p)
        w2 = np.ascontiguousarray(w.transpose(0, 2, 1, 3)).reshape(L, C, HD)
        return np.ascontiguousarray(
            w2.reshape(L, NC, P, HD).transpose(0, 2, 1, 3)).astype(_BF)

    shared = {}
    for nm, key in (("wq", "sa_wq"), ("wk", "sa_wk"), ("wv", "sa_wv"),
                    ("cq", "ca_wq"), ("ck", "ca_wk"), ("cv", "ca_wv")):
        shared[nm] = packw(inputs[key])
    shared["w1"] = np.ascontiguousarray(
        inputs["ff_w1"].reshape(L, NC, P, FF).transpose(0, 2, 1, 3)).astype(_BF)
    shared["w2"] = np.ascontiguousarray(
        inputs["ff_w2"].reshape(L, NF, P, C).transpose(0, 2, 1, 3)).astype(_BF)

    bqk = np.zeros((P, L * 16), np.float32)
    for l in range(L):
        for mi, (bias, s) in enumerate((
                (inputs["sa_bq"][l], 0.125), (inputs["sa_bk"][l], 1.0),
                (inputs["ca_bq"][l], 0.125), (inputs["ca_bk"][l], 1.0))):
            flat = bias.reshape(HD).astype(np.float32) * s
            for mc in range(NC):
                bqk[:, (l * 4 + mi) * 4 + mc] = flat[mc * P:(mc + 1) * P]
    shared["bqk"] = bqk

    bv = np.stack([inputs["sa_bv"].reshape(L, HD),
                   inputs["ca_bv"].reshape(L, HD)], axis=1).astype(np.float32)
    shared["bvb"] = np.ascontiguousarray(
        np.broadcast_to(bv[:, :, None, :], (L, 2, P, HD)))

    b1 = np.zeros((P, L * NF), np.float32)
    for l in range(L):
        for fc in range(NF):
            b1[:, l * NF + fc] = inputs["ff_b1"][l, fc * P:(fc + 1) * P]
    shared["b1"] = b1
    b2 = np.zeros((P, L * NC), np.float32)
    for l in range(L):
        for mc in range(NC):
            b2[:, l * NC + mc] = inputs["ff_b2"][l, mc * P:(mc + 1) * P]
    shared["b2"] = b2
    return shared


LAST_RESULT = None


def _install_ntff_hook():
    """Register the axon NTFF profile hook that the image's antenv lacks.

    Only used for local benchmarking (KERNEL_TRACE=1); inert otherwise.
    """
    import sys
    import types
    try:
        import antenv
        if getattr(antenv, "axon_hooks", None) is not None:
            return
        from trn_agent_boot.trn_boot import _ntff_profile_via_ctypes
        mod = types.ModuleType("antenv.axon_hooks")
        mod._hook = _ntff_profile_via_ctypes("/opt/axon/libaxon_pjrt.so")

        def get_axon_ntff_profile_hook():
            return mod._hook

        def set_axon_ntff_profile_hook(h):
            mod._hook = h

        mod.get_axon_ntff_profile_hook = get_axon_ntff_profile_hook
        mod.set_axon_ntff_profile_hook = set_axon_ntff_profile_hook
        sys.modules["antenv.axon_hooks"] = mod
        antenv.axon_hooks = mod
    except Exception as e:  # pragma: no cover - profiling is best-effort
        print(f"ntff hook install failed: {e}")


def kernel(**inputs):
    global LAST_RESULT
    import os
    inputs = {k: np.asarray(v) for k, v in inputs.items()}
    if "nc" not in _cache:
        _cache["nc"] = _build()
    nc = _cache["nc"]

    shared = _prep_shared(inputs)
    x = inputs["x"].astype(np.float32)
    enc = inputs["encoder_output"].astype(np.float32)

    in_maps = []
    for core in range(NCORES):
        sl = slice(core * BL, (core + 1) * BL)
        xT = np.ascontiguousarray(x[sl].transpose(2, 0, 1)).reshape(C, TB)
        eT = np.ascontiguousarray(enc[sl].transpose(2, 0, 1)).reshape(C, TB)
        m = dict(shared)
        m["xT"] = xT
        m["xTb"] = xT.astype(_BF)
        m["eT"] = eT.astype(_BF)
        in_maps.append(m)

    trace = bool(int(os.environ.get("KERNEL_TRACE", "0")))
    if trace:
        _install_ntff_hook()
    from concourse.bass_utils import run_bass_kernel_spmd
    res = run_bass_kernel_spmd(nc, in_maps, list(range(NCORES)), trace=trace,
                               trace_cores=[0])
    LAST_RESULT = res

    out = np.empty((B, T, C), np.float32)
    for core in range(NCORES):
        outT = res.results[core]["outT"]  # [C, TB]
        out[core * BL:(core + 1) * BL] = outT.reshape(C, BL, T).transpose(1, 2, 0)
    return out
